# revision 1
# baseline (speedup 1.0000x reference)
"""Trainium2 Bass kernel for nn_GroupLinearEncoder.

Math (reference):
  h_b = feat_proj(x_b) = BN(einsum over l,c of x_b and w1_b, w2_b)   (N,1024)
  latent = 0.5*(bn(h0)+bn(h1))
  group_pred = (latent @ shared_w.T) @ embed_w.T + embed_b
  subj       = einsum(latent, fc_w[indices]) + b_sel
  out        = group_pred + subj @ embed_w.T + embed_b
             = (latent @ shared_w.T + subj) @ embed_w.T + 2*embed_b

Key algebraic folds used here:
  * group_pred + subj_res share the embed matmul: z = latent@shared_w.T + subj,
    out = z @ embed_w.T + 2*embed_b  -> embed_w is read ONCE.
  * Because every sample belongs to exactly one group, per-core
    cwt_i = shared_w.T + fc_w.T[:, group_i] applied to mask-selected samples
    and AllReduduced over cores yields z directly (shared term included).

Sharding over 8 cores:
  * feat_proj: data-parallel over batch (8 samples/core), AllGather h.
  * z: group-parallel (core i handles group i via sample masks), AllReduce.
  * embed: column-parallel over out_dim (4944 rows/core, padded), concat on host.
"""

import os
import sys

import numpy as np

N, H, P, KE = 64, 1024, 2048, 39548
PT = P // 128
NCORES = 8
NS = N // NCORES            # samples per core
L0, C0 = 257, 1024
L1, C1 = 197, 768
W = 4944                    # embed rows per core (8*4944 = 39552, 4 pad)
BN_EPS = 1e-5

_CACHE = {}


def _build_nc():
    if "/opt/trn_rl_repo" not in sys.path:
        sys.path.insert(0, "/opt/trn_rl_repo")
    import concourse.bass as bass
    import concourse.tile as tile
    from concourse import bacc, mybir
    from contextlib import ExitStack

    f32 = mybir.dt.float32
    bf16 = mybir.dt.bfloat16
    ALU = mybir.AluOpType
    ACTF = mybir.ActivationFunctionType

    nc = bacc.Bacc(num_devices=NCORES)

    KT = H // 128            # 8 k-tiles
    PT = P // 128            # 16 p-tiles
    NB0 = C0 // 128          # 8 c-chunks branch0
    NB1 = C1 // 128          # 6 c-chunks branch1

    x0t = nc.declare_dram_parameter("x0t", [C0, NS, L0], bf16, isOutput=False)
    x1t = nc.declare_dram_parameter("x1t", [C1, NS, L1], bf16, isOutput=False)
    w2_0t = nc.declare_dram_parameter("w2_0t", [C0, H], bf16, isOutput=False)
    w2_1t = nc.declare_dram_parameter("w2_1t", [C1, H], bf16, isOutput=False)
    w1_0 = nc.declare_dram_parameter("w1_0", [H, L0], f32, isOutput=False)
    w1_1 = nc.declare_dram_parameter("w1_1", [H, L1], f32, isOutput=False)
    gam0 = nc.declare_dram_parameter("gam0", [128, 8], f32, isOutput=False)
    bet0 = nc.declare_dram_parameter("bet0", [128, 8], f32, isOutput=False)
    gam1 = nc.declare_dram_parameter("gam1", [128, 8], f32, isOutput=False)
    bet1 = nc.declare_dram_parameter("bet1", [128, 8], f32, isOutput=False)
    cwt = nc.declare_dram_parameter("cwt", [H, P], bf16, isOutput=False)
    fcb = nc.declare_dram_parameter("fcb", [1, P], bf16, isOutput=False)
    maskrow = nc.declare_dram_parameter("maskrow", [1, N], bf16, isOutput=False)
    mask = nc.declare_dram_parameter("mask", [128, N], f32, isOutput=False)
    ewt = nc.declare_dram_parameter("ewt", [P, W], bf16, isOutput=False)
    eb2 = nc.declare_dram_parameter("eb2", [1, W], bf16, isOutput=False)
    out = nc.declare_dram_parameter("out", [N, W], f32, isOutput=True)

    with tile.TileContext(nc) as tc, ExitStack() as stack:
        singles = stack.enter_context(tc.tile_pool(name="singles", bufs=1))
        dpool = stack.enter_context(tc.tile_pool(name="dram", bufs=1, space="DRAM"))
        tpool = stack.enter_context(tc.tile_pool(name="touchp", bufs=2))
        _tn = [0]

        def touch(ap):
            # absorb a DMA's queue semaphores into DVE's vector clock so
            # downstream DVE ops need only engine-local ordering
            _tn[0] += 1
            tt = tpool.tile([ap.shape[0], 1], ap.dtype, tag="touch",
                            name=f"touch{_tn[0]}")
            nc.vector.tensor_copy(out=tt, in_=ap[:, 0:1])

        # --- resident small tensors ---
        h0sb = singles.tile([128, N], f32)       # col = kt*8 + n_local
        h1sb = singles.tile([128, N], f32)
        gam0sb = singles.tile([128, 8], f32)
        bet0sb = singles.tile([128, 8], f32)
        gam1sb = singles.tile([128, 8], f32)
        bet1sb = singles.tile([128, 8], f32)
        masksb = singles.tile([128, N], f32)
        epssb = singles.tile([128, 1], f32)
        nc.sync.dma_start(out=gam0sb, in_=gam0[:, :])
        nc.sync.dma_start(out=bet0sb, in_=bet0[:, :])
        nc.sync.dma_start(out=gam1sb, in_=gam1[:, :])
        nc.sync.dma_start(out=bet1sb, in_=bet1[:, :])
        nc.sync.dma_start(out=masksb, in_=mask[:, :])
        for _t in (gam0sb, bet0sb, gam1sb, bet1sb, masksb):
            touch(_t)
        nc.vector.memset(epssb, BN_EPS)

        # combined fc+shared weights, resident through stage C
        cwtp = stack.enter_context(tc.tile_pool(name="cwtp", bufs=1))
        cwsb = []
        for kt in range(KT):
            t = cwtp.tile([128, P], bf16, tag=f"cw{kt}", name=f"cw{kt}")
            nc.sync.dma_start(out=t, in_=cwt[kt * 128:(kt + 1) * 128, :])
            cwsb.append(t)

        ps_ctx = tc.tile_pool(name="ps", bufs=2, space="PSUM")
        pspool = ps_ctx.__enter__()

        # ---------------- stage A : feat_proj matmuls ----------------
        # branch 0: per-sample moving operand (N=257 >= 256 keeps f32r fast)
        with tc.tile_pool(name="br0", bufs=1) as br0:
            x0sb = []
            for ci in range(NB0):
                t = br0.tile([128, NS, L0], bf16, tag=f"x0_{ci}", name=f"x0_{ci}")
                nc.sync.dma_start(out=t, in_=x0t[ci * 128:(ci + 1) * 128, :, :])
                x0sb.append(t)
            w1sb = []
            for kt in range(KT):
                t = br0.tile([128, L0], f32, tag=f"w10_{kt}", name=f"w10_{kt}")
                nc.sync.dma_start(out=t, in_=w1_0[kt * 128:(kt + 1) * 128, :])
                touch(t)
                w1sb.append(t)

            for kt in range(KT):
                w2blk = []
                for ci in range(NB0):
                    t = br0.tile([128, 128], bf16, tag=f"w2b{ci}", bufs=2,
                                 name=f"w20b_{kt}_{ci}")
                    nc.sync.dma_start(
                        out=t, in_=w2_0t[ci * 128:(ci + 1) * 128,
                                         kt * 128:(kt + 1) * 128])
                    w2blk.append(t)
                for grp in range(2):
                    vs = []
                    for j in range(4):
                        v = pspool.tile([128, L0], f32, tag=f"v{j}", name=f"v0_{kt}_{grp}_{j}")
                        vs.append(v)
                    for ci in range(NB0):
                        lhs = w2blk[ci][:, :]
                        for j in range(4):
                            n = grp * 4 + j
                            nc.tensor.matmul(
                                out=vs[j][:, :],
                                lhsT=lhs,
                                rhs=x0sb[ci][:, n, :],
                                start=(ci == 0),
                                stop=(ci == NB0 - 1),
                            )
                    for j in range(4):
                        n = grp * 4 + j
                        col = kt * 8 + n
                        nc.vector.tensor_mul(vs[j][:, :], vs[j][:, :], w1sb[kt][:, :])
                        nc.vector.tensor_reduce(
                            out=h0sb[:, col:col + 1], in_=vs[j][:, :],
                            axis=mybir.AxisListType.X, op=ALU.add)

        # branch 1: two samples per moving operand (N=394 >= 256)
        with tc.tile_pool(name="br1", bufs=1) as br1:
            x1sb = []
            for ci in range(NB1):
                t = br1.tile([128, NS, L1], bf16, tag=f"x1_{ci}", name=f"x1_{ci}")
                nc.sync.dma_start(out=t, in_=x1t[ci * 128:(ci + 1) * 128, :, :])
                x1sb.append(t)
            w1sb1 = []
            for kt in range(KT):
                t = br1.tile([128, L1], f32, tag=f"w11_{kt}", name=f"w11_{kt}")
                nc.sync.dma_start(out=t, in_=w1_1[kt * 128:(kt + 1) * 128, :])
                touch(t)
                w1sb1.append(t)

            for kt in range(KT):
                w2blk1 = []
                for ci in range(NB1):
                    t = br1.tile([128, 128], bf16, tag=f"w2c{ci}", bufs=2,
                                 name=f"w21b_{kt}_{ci}")
                    nc.sync.dma_start(
                        out=t, in_=w2_1t[ci * 128:(ci + 1) * 128,
                                         kt * 128:(kt + 1) * 128])
                    w2blk1.append(t)
                for grp in range(2):
                    vps = []
                    for j in range(2):
                        v = pspool.tile([128, 2, L1], f32, tag=f"v{j}", name=f"v1_{kt}_{grp}_{j}")
                        vps.append(v)
                    for ci in range(NB1):
                        lhs = w2blk1[ci][:, :]
                        for j in range(2):
                            pj = grp * 2 + j
                            nc.tensor.matmul(
                                out=vps[j][:, :, :],
                                lhsT=lhs,
                                rhs=x1sb[ci][:, 2 * pj:2 * pj + 2, :],
                                start=(ci == 0),
                                stop=(ci == NB1 - 1),
                            )
                    for j in range(2):
                        pj = grp * 2 + j
                        for s in range(2):
                            n = 2 * pj + s
                            col = kt * 8 + n
                            nc.vector.tensor_mul(vps[j][:, s, :], vps[j][:, s, :],
                                                 w1sb1[kt][:, :])
                            nc.vector.tensor_reduce(
                                out=h1sb[:, col:col + 1], in_=vps[j][:, s, :],
                                axis=mybir.AxisListType.X, op=ALU.add)

        ps_ctx.__exit__(None, None, None)

        # ---------------- stage B : AllGather h + BatchNorm + latent ----------------
        hb_local = dpool.tile([2, 128, N], f32)
        nc.sync.dma_start(out=hb_local[0], in_=h0sb[:, :])
        nc.sync.dma_start(out=hb_local[1], in_=h1sb[:, :])
        hg = dpool.tile([NCORES, 2, 128, N], f32, addr_space="Shared")
        nc.gpsimd.collective_compute(
            "AllGather",
            ALU.bypass,
            replica_groups=[list(range(NCORES))],
            ins=[hb_local[:].opt()],
            outs=[hg[:].opt()],
        )

        # load gathered h: [128, core, branch, 64]
        hall = singles.tile([128, NCORES, 2, N], f32)
        for g in range(NCORES):
            for b in range(2):
                nc.sync.dma_start(out=hall[:, g, b, :], in_=hg[g, b, :, :])
                touch(hall[:, g, b, :])

        latsb = []      # per kt: [128, 64], col = sample g (global)
        stx = stack.enter_context(tc.tile_pool(name="stats", bufs=4))
        for kt in range(KT):
            lat = singles.tile([128, N], f32, tag=f"lat{kt}", name=f"lat{kt}")
            ab = []  # per-branch (a, negb) columns
            for b, (gsb, bsb) in enumerate(((gam0sb, bet0sb), (gam1sb, bet1sb))):
                hga = hall[:, :, b, kt * 8:(kt + 1) * 8]    # [128, g(8), n(8)]
                st = stx.tile([128, NCORES, 6], f32, tag="st", name=f"st{kt}_{b}")
                for g in range(NCORES):
                    nc.vector.bn_stats(out=st[:, g, :], in_=hga[:, g, :])
                mv = stx.tile([128, 2], f32, tag="mv", name=f"mv{kt}_{b}")
                nc.vector.bn_aggr(out=mv, in_=st)
                rst = stx.tile([128, 1], f32, tag="rst", name=f"rst{kt}_{b}")
                # rstd = 1/sqrt(var + eps)
                nc.scalar.activation(out=rst, in_=mv[:, 1:2], func=ACTF.Sqrt,
                                     bias=epssb, scale=1.0)
                nc.vector.reciprocal(out=rst, in_=rst)
                a = stx.tile([128, 1], f32, tag="a", name=f"a{kt}_{b}")
                nc.vector.tensor_mul(a, rst, gsb[:, kt:kt + 1])
                negb = stx.tile([128, 1], f32, tag="negb", name=f"negb{kt}_{b}")
                # negb = mu*a - beta_half
                nc.vector.tensor_mul(negb, mv[:, 0:1], a)
                nc.vector.tensor_sub(negb, negb, bsb[:, kt:kt + 1])
                ab.append((a, negb))
            nbsum = stx.tile([128, 1], f32, tag="nbsum", name=f"nbsum{kt}")
            nc.vector.tensor_add(nbsum, ab[0][1], ab[1][1])
            lat3 = lat[:].rearrange("p (g n) -> p g n", g=NCORES)
            scr = stx.tile([128, NCORES, 8], f32, tag="scr", name=f"scr{kt}")
            # lat = h0*a0 - nbsum
            nc.vector.tensor_scalar_mul(lat3, hall[:, :, 0, kt * 8:(kt + 1) * 8],
                                        ab[0][0][:, 0:1])
            nc.vector.tensor_scalar_sub(lat3, lat3, nbsum[:, 0:1])
            # lat += h1*a1
            nc.vector.tensor_scalar_mul(scr, hall[:, :, 1, kt * 8:(kt + 1) * 8],
                                        ab[1][0][:, 0:1])
            nc.vector.tensor_add(lat3, lat3, scr[:, :, :])
            latsb.append(lat)

        # ---------------- stage C : z partial = cwt.T @ (latent*mask) + fcb x mask ----------------
        lmsb = []
        for kt in range(KT):
            lm = singles.tile([128, N], bf16, tag=f"lm{kt}", name=f"lm{kt}")
            nc.vector.tensor_mul(lm, latsb[kt], masksb)
            lmsb.append(lm)

        zfc_local = dpool.tile([PT, 128, N], f32)
        with tc.tile_pool(name="zps", bufs=1, space="PSUM") as zps, \
             tc.tile_pool(name="csing", bufs=1) as csing:
            fcbsb = csing.tile([1, P], bf16)
            nc.sync.dma_start(out=fcbsb, in_=fcb[:, :])
            maskrsb = csing.tile([1, N], bf16)
            nc.sync.dma_start(out=maskrsb, in_=maskrow[:, :])
            zp = []
            for half in range(2):
                t = zps.tile([128, 8 * N], f32, tag=f"zp{half}", name=f"zp{half}")
                zp.append(t)
            for pt in range(PT):
                o = zp[pt // 8][:, (pt % 8) * N:(pt % 8 + 1) * N]
                for kt in range(KT):
                    nc.tensor.matmul(
                        out=o,
                        lhsT=cwsb[kt][:, pt * 128:(pt + 1) * 128],
                        rhs=lmsb[kt][:, :],
                        start=(kt == 0), stop=False,
                    )
                nc.tensor.matmul(
                    out=o, lhsT=fcbsb[:, pt * 128:(pt + 1) * 128],
                    rhs=maskrsb[:, :], start=False, stop=True)
            for half in range(2):
                zsbuf = csing.tile([128, 8 * N], f32, tag=f"zst{half}", name=f"zst{half}")
                nc.vector.tensor_copy(out=zsbuf, in_=zp[half][:, :])
                src = zsbuf[:].rearrange("p (t c) -> p t c", t=8)
                dst = zfc_local[half * 8:(half + 1) * 8].rearrange("t p c -> p t c")
                nc.sync.dma_start(out=dst, in_=src)

        zr = dpool.tile([PT, 128, N], f32, addr_space="Shared")
        nc.gpsimd.collective_compute(
            "AllReduce",
            ALU.add,
            replica_groups=[list(range(NCORES))],
            ins=[zfc_local[:].opt()],
            outs=[zr[:].opt()],
        )

        zsb = []
        for pt in range(PT):
            t = singles.tile([128, N], f32, tag=f"z{pt}", name=f"z{pt}")
            nc.sync.dma_start(out=t, in_=zr[pt, :, :])
            touch(t)
            tb = singles.tile([128, N], bf16, tag=f"zb{pt}", name=f"zb{pt}")
            nc.vector.tensor_copy(out=tb, in_=t)
            zsb.append(tb)

        # ---------------- stage D : out = z.T @ ewt + 2*eb ----------------
        NBLK = (W + 511) // 512
        with tc.tile_pool(name="ewp", bufs=2) as ewp, \
             tc.tile_pool(name="odp", bufs=2, space="PSUM") as odp, \
             tc.tile_pool(name="osp", bufs=3) as osp, \
             tc.tile_pool(name="dsing", bufs=1) as dsing:
            eb2sb = dsing.tile([1, W], bf16)
            nc.sync.dma_start(out=eb2sb, in_=eb2[:, :])
            ones1 = dsing.tile([1, N], bf16)
            nc.vector.memset(ones1, 1.0)
            for nb in range(NBLK):
                bs = nb * 512
                bw = min(512, W - bs)
                ewtiles = []
                for pc in range(PT):
                    t = ewp.tile([128, 512], bf16, tag=f"ew{pc}", name=f"ew{nb}_{pc}")
                    nc.sync.dma_start(out=t[:, :bw], in_=ewt[pc * 128:(pc + 1) * 128, bs:bs + bw])
                    ewtiles.append(t)
                od = odp.tile([N, 512], f32, tag="od", name=f"od{nb}")
                for pc in range(PT):
                    nc.tensor.matmul(
                        out=od[:, :bw],
                        lhsT=zsb[pc][:, :],
                        rhs=ewtiles[pc][:, :bw],
                        start=(pc == 0), stop=False,
                    )
                nc.tensor.matmul(
                    out=od[:, :bw],
                    lhsT=ones1[:, :],
                    rhs=eb2sb[:, bs:bs + bw],
                    start=False, stop=True,
                )
                osb = osp.tile([N, 512], f32, tag="osb", name=f"osb{nb}")
                nc.vector.tensor_copy(out=osb[:, :bw], in_=od[:, :bw])
                nc.sync.dma_start(out=out[:, bs:bs + bw], in_=osb[:, :bw])

    nc.compile()
    return nc


def _host_prep(x0, x1, w1_0, w2_0, gamma0, beta0, w1_1, w2_1, gamma1, beta1,
               shared_w, fc_w, fc_b, embed_w, embed_b, indices):
    import ml_dtypes
    f = np.float32
    bf = ml_dtypes.bfloat16
    x0t = np.ascontiguousarray(x0.transpose(2, 0, 1)).astype(bf)   # [1024, 64, 257]
    x1t = np.ascontiguousarray(x1.transpose(2, 0, 1)).astype(bf)   # [768, 64, 197]
    w2_0t = np.ascontiguousarray(w2_0.T).astype(bf)
    w2_1t = np.ascontiguousarray(w2_1.T).astype(bf)
    gam0 = np.ascontiguousarray((gamma0 * 0.5).reshape(8, 128).T, dtype=f)
    bet0 = np.ascontiguousarray((beta0 * 0.5).reshape(8, 128).T, dtype=f)
    gam1 = np.ascontiguousarray((gamma1 * 0.5).reshape(8, 128).T, dtype=f)
    bet1 = np.ascontiguousarray((beta1 * 0.5).reshape(8, 128).T, dtype=f)
    swt = shared_w.T.astype(f)                                    # [1024, 2048]
    fcwt = fc_w.T.astype(f)                                       # [1024, 16384]
    ewt_pad = np.zeros((P, NCORES * W), dtype=bf)
    ewt_pad[:, :KE] = embed_w.T.astype(bf)
    eb2_pad = np.zeros((1, NCORES * W), dtype=bf)
    eb2_pad[0, :KE] = (2.0 * embed_b).astype(bf)

    idx = np.asarray(indices).astype(np.int64)
    in_maps = []
    for i in range(NCORES):
        m = (idx == i).astype(f)
        in_maps.append({
            "x0t": np.ascontiguousarray(x0t[:, i * NS:(i + 1) * NS, :]),
            "x1t": np.ascontiguousarray(x1t[:, i * NS:(i + 1) * NS, :]),
            "w2_0t": w2_0t,
            "w2_1t": w2_1t,
            "w1_0": np.ascontiguousarray(w1_0, dtype=f),
            "w1_1": np.ascontiguousarray(w1_1, dtype=f),
            "gam0": gam0, "bet0": bet0, "gam1": gam1, "bet1": bet1,
            "cwt": np.ascontiguousarray(swt + fcwt[:, i * P:(i + 1) * P]).astype(bf),
            "fcb": np.ascontiguousarray(fc_b[i * P:(i + 1) * P].reshape(1, P)).astype(bf),
            "maskrow": np.ascontiguousarray(m.reshape(1, N)).astype(bf),
            "mask": np.ascontiguousarray(np.broadcast_to(m, (128, N))),
            "ewt": np.ascontiguousarray(ewt_pad[:, i * W:(i + 1) * W]),
            "eb2": np.ascontiguousarray(eb2_pad[:, i * W:(i + 1) * W]),
        })
    return in_maps


def kernel(**inputs):
    if "/opt/trn_rl_repo" not in sys.path:
        sys.path.insert(0, "/opt/trn_rl_repo")
    from concourse.bass_utils import run_bass_kernel_spmd

    in_maps = _host_prep(**inputs)
    if "nc" not in _CACHE:
        _CACHE["nc"] = _build_nc()
    nc = _CACHE["nc"]
    res = run_bass_kernel_spmd(nc, in_maps, core_ids=list(range(NCORES)))
    outs = [np.asarray(res.results[i]["out"]) for i in range(NCORES)]
    full = np.concatenate(outs, axis=1)[:, :KE]
    return np.ascontiguousarray(full, dtype=np.float32)


if __name__ == "__main__":
    sys.path.insert(0, os.path.dirname(os.path.abspath(__file__)))
    import reference
    inputs = {k: np.asarray(v) for k, v in reference.setup_inputs().items()}
    expected = np.asarray(reference.reference(**inputs))
    actual = kernel(**inputs)
    err = np.abs(actual - expected).max() / (np.abs(expected).max() + 1e-12)
    print("Relative error:", err)



# revision 7
# speedup vs baseline: 1.5669x; 1.5669x over previous
"""Trainium2 Bass kernel for nn_GroupLinearEncoder.

Math (reference):
  h_b = feat_proj(x_b) = BN(einsum over l,c of x_b and w1_b, w2_b)   (N,1024)
  latent = 0.5*(bn(h0)+bn(h1))
  out = (latent @ shared_w.T + subj) @ embed_w.T + 2*embed_b
  where subj = einsum(latent, fc_w[indices]) + b_sel.

Key algebraic folds:
  * group_pred + subj_res share the embed matmul: z = latent@shared_w.T + subj,
    out = z @ embed_w.T + 2*embed_b  -> embed_w is read ONCE.
  * Because every sample belongs to exactly one group, per-core
    cwt_i = shared_w.T + fc_w.T[:, group_i] applied to mask-selected samples
    and AllReduced over cores yields z directly (shared term included).

Sharding over 8 cores:
  * feat_proj: data-parallel over batch (8 samples/core), AllGather h.
  * z: group-parallel (core i handles group i via sample masks), AllReduce (bf16).
  * embed: column-parallel over out_dim (4944 rows/core, padded), concat on host.

Performance structure (vs v1):
  * embed_w (20.3MB/core) is mostly SBUF-resident: 6 p-tiles prefetched during
    stage A, 6 more during the AllGather/BN/C/AllReduce gap (into SBUF freed by
    the x pools), 4 streamed per 512-col block during stage D.
  * stage D matmuls packed 2x via PE column-group tiling (out partitions 0:64
    and 64:128 run concurrently), recovering the M=64 half-array loss.
  * stage A DVE work batched: one mul+reduce per 4-sample PSUM quad tile with
    stride-0 broadcast w1, instead of per-sample ops.
  * BN stats batched per (branch, ktile) via one bn_stats/bn_aggr pair; BN
    affine applied on the Scalar engine (activation scale/bias APs).
  * z AllReduce in bf16 (exact: per column only one core contributes nonzeros).
"""

import os
import sys

import numpy as np

N, H, P, KE = 64, 1024, 2048, 39548
NCORES = 8
NS = N // NCORES            # samples per core
L0, C0 = 257, 1024
L1, C1 = 197, 768
W = 4944                    # embed rows per core (8*4944 = 39552, 4 pad)
PT = P // 128               # 16
KT = H // 128               # 8
NB0 = C0 // 128             # 8
NB1 = C1 // 128             # 6
BN_EPS = 1e-5
EWA = 6                     # ewt p-tiles resident from stage A
EWB = 6                     # ewt p-tiles loaded during the collective gap
NBLK = (W + 511) // 512     # 10 output blocks

_CACHE = {}


def _build_nc():
    if "/opt/trn_rl_repo" not in sys.path:
        sys.path.insert(0, "/opt/trn_rl_repo")
    import concourse.bass as bass
    import concourse.tile as tile
    from concourse import bacc, mybir
    from contextlib import ExitStack

    f32 = mybir.dt.float32
    bf16 = mybir.dt.bfloat16
    ALU = mybir.AluOpType
    ACTF = mybir.ActivationFunctionType
    X = mybir.AxisListType.X

    nc = bacc.Bacc(num_devices=NCORES)

    x0t = nc.declare_dram_parameter("x0t", [C0, NS, L0], bf16, isOutput=False)
    x1t = nc.declare_dram_parameter("x1t", [C1, NS, L1], bf16, isOutput=False)
    # w2 transposed + kt-paired: [KT//2, C, 256] so each DMA line is 512B
    w2_0p = nc.declare_dram_parameter("w2_0p", [KT // 2, C0, 256], bf16, isOutput=False)
    w2_1p = nc.declare_dram_parameter("w2_1p", [KT // 2, C1, 256], bf16, isOutput=False)
    w1_0 = nc.declare_dram_parameter("w1_0", [H, L0], f32, isOutput=False)
    w1_1 = nc.declare_dram_parameter("w1_1", [H, L1], f32, isOutput=False)
    gb = nc.declare_dram_parameter("gb", [128, 2, KT], f32, isOutput=False)
    bt = nc.declare_dram_parameter("bt", [128, 2, KT], f32, isOutput=False)
    cwt = nc.declare_dram_parameter("cwt", [H, P], bf16, isOutput=False)
    fcb = nc.declare_dram_parameter("fcb", [1, P], bf16, isOutput=False)
    maskrow = nc.declare_dram_parameter("maskrow", [1, N], bf16, isOutput=False)
    mask = nc.declare_dram_parameter("mask", [128, N], f32, isOutput=False)
    ewt = nc.declare_dram_parameter("ewt", [PT, 128, W], bf16, isOutput=False)
    eb2 = nc.declare_dram_parameter("eb2", [1, W], bf16, isOutput=False)
    out = nc.declare_dram_parameter("out", [N, W], f32, isOutput=True)

    with tile.TileContext(nc) as tc, ExitStack() as stack:
        singles = stack.enter_context(tc.tile_pool(name="singles", bufs=1))
        dpool = stack.enter_context(tc.tile_pool(name="dram", bufs=1, space="DRAM"))
        tpool = stack.enter_context(tc.tile_pool(name="touchp", bufs=2))
        _tn = [0]

        def touch(ap):
            # absorb a DMA's queue semaphores into DVE's vector clock so
            # downstream DVE ops need only engine-local ordering
            _tn[0] += 1
            tt = tpool.tile([ap.shape[0], 1], ap.dtype, tag="touch",
                            name=f"touch{_tn[0]}")
            nc.vector.tensor_copy(out=tt, in_=ap[:, 0:1])

        # --- persistent small tensors ---
        h0sb = singles.tile([128, N], f32)       # col = kt*8 + n_local
        h1sb = singles.tile([128, N], f32)
        gbsb = singles.tile([128, 2, KT], f32)   # 0.5*gamma, [*, branch, kt]
        btsb = singles.tile([128, 2, KT], f32)   # 0.5*beta
        masksb = singles.tile([128, N], f32)
        epssb = singles.tile([128, 1], f32)
        hall = singles.tile([128, NCORES, 2, N], f32)   # gathered h
        lmsb = []                                 # per kt: latent*mask, bf16
        for kt in range(KT):
            lmsb.append(singles.tile([128, N], bf16, tag=f"lm{kt}", name=f"lm{kt}"))
        zsb_all = singles.tile([128, 2, 8 * N], bf16)    # z after AllReduce
        fcbsb = singles.tile([1, P], bf16)
        maskrsb = singles.tile([1, N], bf16)
        eb2sb = singles.tile([1, W], bf16)
        ones1 = singles.tile([1, N], bf16)
        gate = singles.tile([128, 1], f32)

        cwtp = stack.enter_context(tc.tile_pool(name="cwtp", bufs=1))
        ewpA = stack.enter_context(tc.tile_pool(name="ewpA", bufs=1))

        # ---- stage-A input pool (freed after stage A; space reused by ewpB)
        ctx_brx = tc.tile_pool(name="brx", bufs=1)
        brx = ctx_brx.__enter__()

        # x loads first: they gate the first matmuls
        x0sb = []
        for ci in range(NB0):
            t = brx.tile([128, NS, L0], bf16, tag=f"x0_{ci}", name=f"x0_{ci}")
            nc.sync.dma_start(out=t, in_=x0t[ci * 128:(ci + 1) * 128, :, :])
            x0sb.append(t)
        x1sb = []
        for ci in range(NB1):
            t = brx.tile([128, NS, L1], bf16, tag=f"x1_{ci}", name=f"x1_{ci}")
            nc.sync.dma_start(out=t, in_=x1t[ci * 128:(ci + 1) * 128, :, :])
            x1sb.append(t)
        w1sb0, w1sb1 = [], []
        for kt in range(KT):
            t = brx.tile([128, L0], f32, tag=f"w10_{kt}", name=f"w10_{kt}")
            nc.sync.dma_start(out=t, in_=w1_0[kt * 128:(kt + 1) * 128, :])
            touch(t)
            w1sb0.append(t)
        for kt in range(KT):
            t = brx.tile([128, L1], f32, tag=f"w11_{kt}", name=f"w11_{kt}")
            nc.sync.dma_start(out=t, in_=w1_1[kt * 128:(kt + 1) * 128, :])
            touch(t)
            w1sb1.append(t)

        # small singles (sync queue, after x)
        nc.sync.dma_start(out=gbsb, in_=gb[:, :, :])
        nc.sync.dma_start(out=btsb, in_=bt[:, :, :])
        nc.sync.dma_start(out=masksb, in_=mask[:, :])
        nc.sync.dma_start(out=fcbsb, in_=fcb[:, :])
        nc.sync.dma_start(out=maskrsb, in_=maskrow[:, :])
        touch(gbsb[:].rearrange("p a b -> p (a b)"))
        touch(btsb[:].rearrange("p a b -> p (a b)"))
        touch(masksb)
        nc.vector.memset(epssb, BN_EPS)
        nc.vector.memset(ones1, 1.0)

        psA_ctx = tc.tile_pool(name="psA", bufs=2, space="PSUM")
        psA = psA_ctx.__enter__()

        # ---------------- stage A : feat_proj ----------------
        cwsb = []
        ewsb = [None] * PT
        w2blk = {}

        def _load_w2_pair(branch, kp):
            # one [128, 2, 128] tile per ci holding kt=2*kp and 2*kp+1 slices
            src = w2_0p if branch == 0 else w2_1p
            nb = NB0 if branch == 0 else NB1
            blks = []
            for ci in range(nb):
                t = brx.tile([128, 2, 128], bf16, tag=f"w2_{branch}_{ci}", bufs=2,
                             name=f"w2_{branch}_{kp}_{ci}")
                nc.sync.dma_start(
                    out=t, in_=src[kp, ci * 128:(ci + 1) * 128, :].rearrange(
                        "c (j u) -> c j u", j=2))
                blks.append(t)
            return blks

        # branch 0: 4-sample PSUM quads; one DVE mul+reduce per quad
        for kt in range(KT):
            if kt % 2 == 0:
                w2blk[0] = _load_w2_pair(0, kt // 2)
            for grp in range(2):
                quad = psA.tile([128, 4, 512], f32, tag="quad",
                                name=f"q0_{kt}_{grp}")
                for ci in range(NB0):
                    lhs = w2blk[0][ci][:, kt % 2, :]
                    for j in range(4):
                        n = grp * 4 + j
                        nc.tensor.matmul(
                            out=quad[:, j, 0:L0],
                            lhsT=lhs,
                            rhs=x0sb[ci][:, n, :],
                            start=(ci == 0),
                            stop=(ci == NB0 - 1),
                        )
                w1b = w1sb0[kt][:].rearrange("p (o l) -> p o l", o=1) \
                    .to_broadcast([128, 4, L0])
                nc.vector.tensor_mul(quad[:, :, 0:L0], quad[:, :, 0:L0], w1b)
                c0 = kt * 8 + grp * 4
                nc.vector.tensor_reduce(
                    out=h0sb[:, c0:c0 + 4], in_=quad[:, :, 0:L0],
                    axis=X, op=ALU.add)
            if kt == 2:
                # gate bulk scalar-queue loads behind early stage-A output so
                # x/w2 win HBM bandwidth during startup
                nc.scalar.activation(out=gate, in_=h0sb[:, 23:24], func=ACTF.Copy)
                for kt2 in range(KT):
                    t = cwtp.tile([128, P], bf16, tag=f"cw{kt2}", name=f"cw{kt2}")
                    nc.scalar.dma_start(out=t, in_=cwt[kt2 * 128:(kt2 + 1) * 128, :])
                    cwsb.append(t)
                for pc in range(EWA):
                    t = ewpA.tile([128, W], bf16, tag=f"ewA{pc}", name=f"ewA{pc}")
                    nc.scalar.dma_start(out=t, in_=ewt[pc, :, :])
                    ewsb[pc] = t
                nc.scalar.dma_start(out=eb2sb, in_=eb2[:, :])

        # branch 1: two samples per matmul; one DVE mul+reduce per kt
        for kt in range(KT):
            if kt % 2 == 0:
                w2blk[1] = _load_w2_pair(1, kt // 2)
            quad = psA.tile([128, 4, 512], f32, tag="quad", name=f"q1_{kt}")
            for ci in range(NB1):
                lhs = w2blk[1][ci][:, kt % 2, :]
                for sj in range(4):
                    o = quad[:, sj, :].rearrange("p (s l) -> p s l", s=2)
                    nc.tensor.matmul(
                        out=o[:, :, 0:L1],
                        lhsT=lhs,
                        rhs=x1sb[ci][:, 2 * sj:2 * sj + 2, :],
                        start=(ci == 0),
                        stop=(ci == NB1 - 1),
                    )
            v = quad[:].rearrange("p q (s l) -> p q s l", s=2)[:, :, :, 0:L1]
            w1b = w1sb1[kt][:].rearrange("p (a b l) -> p a b l", a=1, b=1) \
                .to_broadcast([128, 4, 2, L1])
            nc.vector.tensor_mul(v, v, w1b)
            nc.vector.tensor_reduce(
                out=h1sb[:, kt * 8:(kt + 1) * 8], in_=v, axis=X, op=ALU.add)

        psA_ctx.__exit__(None, None, None)
        ctx_brx.__exit__(None, None, None)

        # remaining embed weight tiles into the space freed by the x pools;
        # the DMAs wait on the last stage-A reads, then flow during the
        # AllGather / BN / stage-C / AllReduce gap.
        ewpB = stack.enter_context(tc.tile_pool(name="ewpB", bufs=1))
        for pc in range(EWA, EWA + EWB):
            t = ewpB.tile([128, W], bf16, tag=f"ewB{pc}", name=f"ewB{pc}")
            nc.scalar.dma_start(out=t, in_=ewt[pc, :, :])
            ewsb[pc] = t

        # ---------------- stage B : AllGather h + BatchNorm + latent ----------------
        hb_local = dpool.tile([2, 128, N], f32)
        nc.sync.dma_start(out=hb_local[0], in_=h0sb[:, :])
        nc.sync.dma_start(out=hb_local[1], in_=h1sb[:, :])
        hg = dpool.tile([NCORES, 2, 128, N], f32, addr_space="Shared")
        nc.gpsimd.collective_compute(
            "AllGather",
            ALU.bypass,
            replica_groups=[list(range(NCORES))],
            ins=[hb_local[:].opt()],
            outs=[hg[:].opt()],
        )

        # load gathered h in one DMA: hall[p, g, b, col]
        nc.sync.dma_start(
            out=hall[:].rearrange("p g b n -> p (g b) n"),
            in_=hg[:].rearrange("g b p n -> p (g b) n"))
        touch(hall[:, 0, 0, :])

        stx = stack.enter_context(tc.tile_pool(name="stats", bufs=2))
        XY = mybir.AxisListType.XY
        hsq = singles.tile([128, NCORES, 2, N], f32)
        sums = singles.tile([128, 2, KT], f32)     # becomes mean
        sumq = singles.tile([128, 2, KT], f32)     # becomes var
        tmp_ = singles.tile([128, 2, KT], f32)
        nc.vector.tensor_mul(hsq[:].rearrange("p g b n -> p (g b n)"),
                             hall[:].rearrange("p g b n -> p (g b n)"),
                             hall[:].rearrange("p g b n -> p (g b n)"))
        for b in range(2):
            for kt in range(KT):
                nc.vector.tensor_reduce(
                    out=sums[:, b, kt:kt + 1],
                    in_=hall[:, :, b, kt * 8:(kt + 1) * 8], axis=XY, op=ALU.add)
                nc.vector.tensor_reduce(
                    out=sumq[:, b, kt:kt + 1],
                    in_=hsq[:, :, b, kt * 8:(kt + 1) * 8], axis=XY, op=ALU.add)
        # mean/var (biased); rstd = 1/sqrt(var+eps); a = 0.5*gamma*rstd;
        # c = 0.5*beta - mu*a
        nc.scalar.activation(out=sums[:], in_=sums[:], func=ACTF.Copy,
                             scale=1.0 / N)
        nc.scalar.activation(out=sumq[:], in_=sumq[:], func=ACTF.Copy,
                             scale=1.0 / N)
        nc.vector.tensor_mul(tmp_[:], sums[:], sums[:])
        nc.vector.tensor_sub(sumq[:], sumq[:], tmp_[:])
        a_ = singles.tile([128, 2, KT], f32)
        c_ = singles.tile([128, 2, KT], f32)
        nc.scalar.activation(out=a_[:], in_=sumq[:], func=ACTF.Sqrt,
                             bias=epssb, scale=1.0)
        nc.vector.reciprocal(out=a_[:].rearrange("p b k -> p (b k)"),
                             in_=a_[:].rearrange("p b k -> p (b k)"))
        nc.vector.tensor_mul(a_[:], a_[:], gbsb[:])
        nc.vector.tensor_mul(c_[:], sums[:], a_[:])
        nc.vector.tensor_sub(c_[:], btsb[:], c_[:])

        # latent = h0*a0+c0 + h1*a1+c1 (per kt); lm = latent*mask (bf16)
        for kt in range(KT):
            latk = stx.tile([128, NCORES, NS], f32, tag="lat", name=f"lat{kt}")
            scrk = stx.tile([128, NCORES, NS], f32, tag="scr", name=f"scr{kt}")
            nc.scalar.activation(out=latk[:], in_=hall[:, :, 0, kt * 8:(kt + 1) * 8],
                                 func=ACTF.Identity,
                                 scale=a_[:, 0, kt:kt + 1], bias=c_[:, 0, kt:kt + 1])
            nc.scalar.activation(out=scrk[:], in_=hall[:, :, 1, kt * 8:(kt + 1) * 8],
                                 func=ACTF.Identity,
                                 scale=a_[:, 1, kt:kt + 1], bias=c_[:, 1, kt:kt + 1])
            lat2 = latk[:].rearrange("p g n -> p (g n)")
            nc.vector.tensor_add(lat2, lat2, scrk[:].rearrange("p g n -> p (g n)"))
            nc.vector.tensor_mul(lmsb[kt][:], lat2, masksb[:])

        # ---------------- stage C : z partial = cwt.T @ lm + fcb x mask ----------------
        zfc_local = dpool.tile([2, 128, 8 * N], bf16)
        with tc.tile_pool(name="zps", bufs=1, space="PSUM") as zps:
            zp = []
            for half in range(2):
                t = zps.tile([128, 8 * N], f32, tag=f"zp{half}", name=f"zp{half}")
                zp.append(t)
            for pt in range(PT):
                o = zp[pt // 8][:, (pt % 8) * N:(pt % 8 + 1) * N]
                for kt in range(KT):
                    nc.tensor.matmul(
                        out=o,
                        lhsT=cwsb[kt][:, pt * 128:(pt + 1) * 128],
                        rhs=lmsb[kt][:, :],
                        start=(kt == 0), stop=False,
                    )
                nc.tensor.matmul(
                    out=o, lhsT=fcbsb[:, pt * 128:(pt + 1) * 128],
                    rhs=maskrsb[:, :], start=False, stop=True)
            for half in range(2):
                zhb = stx.tile([128, 8 * N], bf16, tag=f"zh{half}", name=f"zh{half}")
                nc.vector.tensor_copy(out=zhb, in_=zp[half][:, :])
                nc.sync.dma_start(out=zfc_local[half], in_=zhb)

        zr = dpool.tile([2, 128, 8 * N], bf16, addr_space="Shared")
        nc.gpsimd.collective_compute(
            "AllReduce",
            ALU.add,
            replica_groups=[list(range(NCORES))],
            ins=[zfc_local[:].opt()],
            outs=[zr[:].opt()],
        )
        nc.sync.dma_start(
            out=zsb_all[:],
            in_=zr[:].rearrange("h p c -> p h c"))
        touch(zsb_all[:, 0, :])

        def zview(pc):
            return zsb_all[:, pc // 8, (pc % 8) * N:(pc % 8 + 1) * N]

        # ---------------- stage D : out = z.T @ ewt + 2*eb ----------------
        with tc.tile_pool(name="ewpC", bufs=2) as ewpC, \
             tc.tile_pool(name="odp", bufs=2, space="PSUM") as odp, \
             tc.tile_pool(name="osp", bufs=3) as osp:
            for nb in range(NBLK):
                bs = nb * 512
                bw = min(512, W - bs)
                echunk = {}
                for pc in range(EWA + EWB, PT):
                    t = ewpC.tile([128, 512], bf16, tag=f"s{pc}",
                                  name=f"ewC{nb}_{pc}")
                    nc.scalar.dma_start(out=t[:, :bw], in_=ewt[pc, :, bs:bs + bw])
                    echunk[pc] = t
                odA = odp.tile([128, 512], f32, tag="odA", name=f"odA{nb}")
                odB = odp.tile([128, 512], f32, tag="odB", name=f"odB{nb}")

                def esrc(pc):
                    if ewsb[pc] is not None:
                        return ewsb[pc][:, bs:bs + bw]
                    return echunk[pc][:, :bw]

                # bias into the lower half's accumulator
                nc.tensor.matmul(
                    out=odA[0:64, :bw], lhsT=ones1[:, :], rhs=eb2sb[:, bs:bs + bw],
                    start=True, stop=False, tile_position=(0, 0))
                for pc in range(8):
                    nc.tensor.matmul(
                        out=odA[0:64, :bw], lhsT=zview(pc), rhs=esrc(pc),
                        start=False, stop=(pc == 7), tile_position=(0, 0))
                    nc.tensor.matmul(
                        out=odB[64:128, :bw], lhsT=zview(pc + 8), rhs=esrc(pc + 8),
                        start=(pc == 0), stop=(pc == 7), tile_position=(0, 64))
                osb = osp.tile([64, 512], f32, tag="osb", name=f"osb{nb}")
                nc.scalar.activation(out=osb[:, :bw], in_=odA[0:64, :bw],
                                     func=ACTF.Copy)
                nc.vector.tensor_add(osb[:, :bw], osb[:, :bw], odB[64:128, :bw])
                nc.sync.dma_start(out=out[:, bs:bs + bw], in_=osb[:, :bw])

    nc.compile()
    return nc


def _host_prep(x0, x1, w1_0, w2_0, gamma0, beta0, w1_1, w2_1, gamma1, beta1,
               shared_w, fc_w, fc_b, embed_w, embed_b, indices):
    import ml_dtypes
    f = np.float32
    bf = ml_dtypes.bfloat16
    x0t = np.ascontiguousarray(x0.transpose(2, 0, 1)).astype(bf)   # [1024, 64, 257]
    x1t = np.ascontiguousarray(x1.transpose(2, 0, 1)).astype(bf)   # [768, 64, 197]
    # w2 transposed, columns grouped in kt-pairs: [KT//2, C, 256]
    w2_0t = w2_0.T.astype(bf)                                      # [C0, H]
    w2_1t = w2_1.T.astype(bf)
    w2_0pp = np.ascontiguousarray(
        w2_0t.reshape(C0, KT // 2, 256).transpose(1, 0, 2))
    w2_1pp = np.ascontiguousarray(
        w2_1t.reshape(C1, KT // 2, 256).transpose(1, 0, 2))
    gbp = np.empty((128, 2, KT), f)
    btp = np.empty((128, 2, KT), f)
    gbp[:, 0, :] = (gamma0 * 0.5).reshape(KT, 128).T
    gbp[:, 1, :] = (gamma1 * 0.5).reshape(KT, 128).T
    btp[:, 0, :] = (beta0 * 0.5).reshape(KT, 128).T
    btp[:, 1, :] = (beta1 * 0.5).reshape(KT, 128).T
    swt = shared_w.T.astype(f)                                    # [1024, 2048]
    fcwt = fc_w.T.astype(f)                                       # [1024, 16384]
    ewt_pad = np.zeros((P, NCORES * W), dtype=bf)
    ewt_pad[:, :KE] = embed_w.T.astype(bf)
    eb2_pad = np.zeros((1, NCORES * W), dtype=bf)
    eb2_pad[0, :KE] = (2.0 * embed_b).astype(bf)

    idx = np.asarray(indices).astype(np.int64)
    in_maps = []
    for i in range(NCORES):
        m = (idx == i).astype(f)
        ew_core = ewt_pad[:, i * W:(i + 1) * W]                   # [2048, W]
        in_maps.append({
            "x0t": np.ascontiguousarray(x0t[:, i * NS:(i + 1) * NS, :]),
            "x1t": np.ascontiguousarray(x1t[:, i * NS:(i + 1) * NS, :]),
            "w2_0p": w2_0pp,
            "w2_1p": w2_1pp,
            "w1_0": np.ascontiguousarray(w1_0, dtype=f),
            "w1_1": np.ascontiguousarray(w1_1, dtype=f),
            "gb": gbp, "bt": btp,
            "cwt": np.ascontiguousarray(swt + fcwt[:, i * P:(i + 1) * P]).astype(bf),
            "fcb": np.ascontiguousarray(fc_b[i * P:(i + 1) * P].reshape(1, P)).astype(bf),
            "maskrow": np.ascontiguousarray(m.reshape(1, N)).astype(bf),
            "mask": np.ascontiguousarray(np.broadcast_to(m, (128, N))),
            "ewt": np.ascontiguousarray(
                ew_core.reshape(PT, 128, W)),
            "eb2": np.ascontiguousarray(eb2_pad[:, i * W:(i + 1) * W]),
        })
    return in_maps


def kernel(**inputs):
    if "/opt/trn_rl_repo" not in sys.path:
        sys.path.insert(0, "/opt/trn_rl_repo")
    from concourse.bass_utils import run_bass_kernel_spmd

    in_maps = _host_prep(**inputs)
    if "nc" not in _CACHE:
        _CACHE["nc"] = _build_nc()
    nc = _CACHE["nc"]
    res = run_bass_kernel_spmd(nc, in_maps, core_ids=list(range(NCORES)))
    outs = [np.asarray(res.results[i]["out"]) for i in range(NCORES)]
    full = np.concatenate(outs, axis=1)[:, :KE]
    return np.ascontiguousarray(full, dtype=np.float32)


if __name__ == "__main__":
    sys.path.insert(0, os.path.dirname(os.path.abspath(__file__)))
    import reference
    inputs = {k: np.asarray(v) for k, v in reference.setup_inputs().items()}
    expected = np.asarray(reference.reference(**inputs))
    actual = kernel(**inputs)
    err = np.abs(actual - expected).max() / (np.abs(expected).max() + 1e-12)
    print("Relative error:", err)


# revision 17
# speedup vs baseline: 1.7222x; 1.0991x over previous
"""Trainium2 Bass kernel for nn_GroupLinearEncoder.

Math (reference):
  h_b = feat_proj(x_b) = BN(einsum over l,c of x_b and w1_b, w2_b)   (N,1024)
  latent = 0.5*(bn(h0)+bn(h1))
  out = (latent @ shared_w.T + subj) @ embed_w.T + 2*embed_b
  where subj = einsum(latent, fc_w[indices]) + b_sel.

Key algebraic folds:
  * group_pred + subj_res share the embed matmul: z = latent@shared_w.T + subj,
    out = z @ embed_w.T + 2*embed_b  -> embed_w is read ONCE.
  * Because every sample belongs to exactly one group, per-core
    cwt_i = shared_w.T + fc_w.T[:, group_i] applied to mask-selected samples
    and AllReduced over cores yields z directly (shared term included).

Sharding over 8 cores:
  * feat_proj: data-parallel over batch (8 samples/core), AllGather h.
  * z: group-parallel (core i handles group i via sample masks), AllReduce (bf16).
  * embed: column-parallel over out_dim (4944 rows/core, padded), concat on host.

Performance structure (vs v1):
  * embed_w (20.3MB/core) is mostly SBUF-resident: 6 p-tiles prefetched during
    stage A, 6 more during the AllGather/BN/C/AllReduce gap (into SBUF freed by
    the x pools), 4 streamed per 512-col block during stage D.
  * stage D matmuls packed 2x via PE column-group tiling (out partitions 0:64
    and 64:128 run concurrently), recovering the M=64 half-array loss.
  * stage A DVE work batched: one mul+reduce per 4-sample PSUM quad tile with
    stride-0 broadcast w1, instead of per-sample ops.
  * BN stats batched per (branch, ktile) via one bn_stats/bn_aggr pair; BN
    affine applied on the Scalar engine (activation scale/bias APs).
  * z AllReduce in bf16 (exact: per column only one core contributes nonzeros).
"""

import os
import sys

import numpy as np

N, H, P, KE = 64, 1024, 2048, 39548
NCORES = 8
NS = N // NCORES            # samples per core
L0, C0 = 257, 1024
L1, C1 = 197, 768
W = 4944                    # embed rows per core (8*4944 = 39552, 4 pad)
PT = P // 128               # 16
KT = H // 128               # 8
NB0 = C0 // 128             # 8
NB1 = C1 // 128             # 6
BN_EPS = 1e-5
EWA = 6                     # ewt p-tiles resident from stage A
EWB = 6                     # ewt p-tiles loaded during the collective gap
NBLK = (W + 511) // 512     # 10 output blocks

_CACHE = {}


def _build_nc():
    if "/opt/trn_rl_repo" not in sys.path:
        sys.path.insert(0, "/opt/trn_rl_repo")
    import concourse.bass as bass
    import concourse.tile as tile
    from concourse import bacc, mybir
    from contextlib import ExitStack

    f32 = mybir.dt.float32
    bf16 = mybir.dt.bfloat16
    ALU = mybir.AluOpType
    ACTF = mybir.ActivationFunctionType
    X = mybir.AxisListType.X

    nc = bacc.Bacc(num_devices=NCORES)

    x0t = nc.declare_dram_parameter("x0t", [C0, NS, L0], bf16, isOutput=False)
    x1t = nc.declare_dram_parameter("x1t", [C1, NS, L1], bf16, isOutput=False)
    # w2 transposed + kt-paired: [KT//2, C, 256] so each DMA line is 512B
    w2_0p = nc.declare_dram_parameter("w2_0p", [KT // 2, C0, 256], bf16, isOutput=False)
    w2_1p = nc.declare_dram_parameter("w2_1p", [KT // 2, C1, 256], bf16, isOutput=False)
    w1_0 = nc.declare_dram_parameter("w1_0", [H, L0], f32, isOutput=False)
    w1_1 = nc.declare_dram_parameter("w1_1", [H, L1], f32, isOutput=False)
    gb = nc.declare_dram_parameter("gb", [128, 2, KT], f32, isOutput=False)
    bt = nc.declare_dram_parameter("bt", [128, 2, KT], f32, isOutput=False)
    cwt = nc.declare_dram_parameter("cwt", [H, P], bf16, isOutput=False)
    fcb = nc.declare_dram_parameter("fcb", [1, P], bf16, isOutput=False)
    maskrow = nc.declare_dram_parameter("maskrow", [1, N], bf16, isOutput=False)
    mask = nc.declare_dram_parameter("mask", [128, N], f32, isOutput=False)
    ewt = nc.declare_dram_parameter("ewt", [PT, 128, W], bf16, isOutput=False)
    eb2 = nc.declare_dram_parameter("eb2", [1, W], bf16, isOutput=False)
    out = nc.declare_dram_parameter("out", [N, W], f32, isOutput=True)

    with tile.TileContext(nc) as tc, ExitStack() as stack:
        singles = stack.enter_context(tc.tile_pool(name="singles", bufs=1))
        dpool = stack.enter_context(tc.tile_pool(name="dram", bufs=1, space="DRAM"))
        tpool = stack.enter_context(tc.tile_pool(name="touchp", bufs=2))
        _tn = [0]

        def touch(ap):
            # absorb a DMA's queue semaphores into DVE's vector clock so
            # downstream DVE ops need only engine-local ordering
            _tn[0] += 1
            tt = tpool.tile([ap.shape[0], 1], ap.dtype, tag="touch",
                            name=f"touch{_tn[0]}")
            nc.vector.tensor_copy(out=tt, in_=ap[:, 0:1])

        # --- persistent small tensors ---
        h0sb = singles.tile([128, N], f32)       # col = kt*8 + n_local
        h1sb = singles.tile([128, N], f32)
        gbsb = singles.tile([128, 2, KT], f32)   # 0.5*gamma, [*, branch, kt]
        btsb = singles.tile([128, 2, KT], f32)   # 0.5*beta
        masksb = singles.tile([128, N], f32)
        epssb = singles.tile([128, 1], f32)
        hall = singles.tile([128, NCORES, 2, N], f32)   # gathered h
        lm_all = singles.tile([128, KT, N], bf16)  # latent*mask per ktile
        zsb_all = singles.tile([128, 2, 8 * N], bf16)    # z after AllReduce
        fcbsb = singles.tile([1, P], bf16)
        maskrsb = singles.tile([1, N], bf16)
        eb2sb = singles.tile([1, W], bf16)
        ones1 = singles.tile([1, N], bf16)
        gate = singles.tile([128, 1], f32)

        cwtp = stack.enter_context(tc.tile_pool(name="cwtp", bufs=1))
        ewpA = stack.enter_context(tc.tile_pool(name="ewpA", bufs=1))

        # ---- stage-A input pool (freed after stage A; space reused by ewpB)
        ctx_brx = tc.tile_pool(name="brx", bufs=1)
        brx = ctx_brx.__enter__()

        # DMAs on one HWDGE ring execute FIFO (~175GB/s per ring), so the
        # startup-critical loads are split across the sync and gpsimd rings:
        #   sync:   w2_0 pair 0, x0 even ci  (first-matmul dependencies)
        #   gpsimd: x0 odd ci, x1, w1, small singles
        x0sb = [None] * NB0
        x1sb = []
        w2blk = {}

        def _w2_tile(branch, ci, kp):
            return brx.tile([128, 2, 128], bf16, tag=f"w2_{branch}_{ci}", bufs=2,
                            name=f"w2_{branch}_{kp}_{ci}")

        def _load_w2_pair(branch, kp, eng):
            src = w2_0p if branch == 0 else w2_1p
            nb = NB0 if branch == 0 else NB1
            blks = []
            for ci in range(nb):
                t = _w2_tile(branch, ci, kp)
                eng.dma_start(
                    out=t, in_=src[kp, ci * 128:(ci + 1) * 128, :].rearrange(
                        "c (j u) -> c j u", j=2))
                blks.append(t)
            return blks

        w2blk[0] = _load_w2_pair(0, 0, nc.sync)
        for ci in range(NB0):
            t = brx.tile([128, NS, L0], bf16, tag=f"x0_{ci}", name=f"x0_{ci}")
            eng = nc.sync if ci % 2 == 0 else nc.gpsimd
            eng.dma_start(out=t, in_=x0t[ci * 128:(ci + 1) * 128, :, :])
            x0sb[ci] = t
        for ci in range(NB1):
            t = brx.tile([128, NS, L1], bf16, tag=f"x1_{ci}", name=f"x1_{ci}")
            nc.gpsimd.dma_start(out=t, in_=x1t[ci * 128:(ci + 1) * 128, :, :])
            x1sb.append(t)
        w1sb0, w1sb1 = [], []
        for kt in range(KT):
            t = brx.tile([128, L0], f32, tag=f"w10_{kt}", name=f"w10_{kt}")
            nc.gpsimd.dma_start(out=t, in_=w1_0[kt * 128:(kt + 1) * 128, :])
            touch(t)
            w1sb0.append(t)
        for kt in range(KT):
            t = brx.tile([128, L1], f32, tag=f"w11_{kt}", name=f"w11_{kt}")
            nc.gpsimd.dma_start(out=t, in_=w1_1[kt * 128:(kt + 1) * 128, :])
            touch(t)
            w1sb1.append(t)

        nc.gpsimd.dma_start(out=gbsb, in_=gb[:, :, :])
        nc.gpsimd.dma_start(out=btsb, in_=bt[:, :, :])
        nc.gpsimd.dma_start(out=masksb, in_=mask[:, :])
        nc.gpsimd.dma_start(out=fcbsb, in_=fcb[:, :])
        nc.gpsimd.dma_start(out=maskrsb, in_=maskrow[:, :])
        touch(gbsb[:].rearrange("p a b -> p (a b)"))
        touch(btsb[:].rearrange("p a b -> p (a b)"))
        touch(masksb)
        nc.vector.memset(epssb, BN_EPS)
        nc.vector.memset(ones1, 1.0)

        psA_ctx = tc.tile_pool(name="psA", bufs=2, space="PSUM")
        psA = psA_ctx.__enter__()

        # ---------------- stage A : feat_proj ----------------
        cwsb = []
        ewsb = [None] * PT

        # branch 0: 4-sample PSUM quads; one DVE mul+reduce per quad
        for kt in range(KT):
            if kt % 2 == 0 and kt > 0:
                w2blk[0] = _load_w2_pair(0, kt // 2, nc.sync)
            for grp in range(2):
                quad = psA.tile([128, 4, 512], f32, tag="quad",
                                name=f"q0_{kt}_{grp}")
                for ci in range(NB0):
                    lhs = w2blk[0][ci][:, kt % 2, :]
                    for j in range(4):
                        n = grp * 4 + j
                        nc.tensor.matmul(
                            out=quad[:, j, 0:L0],
                            lhsT=lhs,
                            rhs=x0sb[ci][:, n, :],
                            start=(ci == 0),
                            stop=(ci == NB0 - 1),
                        )
                w1b = w1sb0[kt][:].rearrange("p (o l) -> p o l", o=1) \
                    .to_broadcast([128, 4, L0])
                nc.vector.tensor_mul(quad[:, :, 0:L0], quad[:, :, 0:L0], w1b)
                c0 = kt * 8 + grp * 4
                nc.vector.tensor_reduce(
                    out=h0sb[:, c0:c0 + 4], in_=quad[:, :, 0:L0],
                    axis=X, op=ALU.add)
            if kt == 2:
                # gate bulk scalar-queue loads behind early stage-A output so
                # x/w2 win HBM bandwidth during startup
                nc.scalar.activation(out=gate, in_=h0sb[:, 23:24], func=ACTF.Copy)
                for kt2 in range(KT):
                    t = cwtp.tile([128, P], bf16, tag=f"cw{kt2}", name=f"cw{kt2}")
                    nc.scalar.dma_start(out=t, in_=cwt[kt2 * 128:(kt2 + 1) * 128, :])
                    cwsb.append(t)
                for pc in range(EWA):
                    t = ewpA.tile([128, W], bf16, tag=f"ewA{pc}", name=f"ewA{pc}")
                    nc.scalar.dma_start(out=t, in_=ewt[pc, :, :])
                    ewsb[pc] = t
                nc.scalar.dma_start(out=eb2sb, in_=eb2[:, :])

        # h0 store early so the AllGather input is half-ready before br1 ends
        hb_local = dpool.tile([2, 128, N], f32)
        nc.sync.dma_start(out=hb_local[0], in_=h0sb[:, :])

        # branch 1: two samples per matmul; one DVE mul+reduce per kt
        for kt in range(KT):
            if kt % 2 == 0:
                w2blk[1] = _load_w2_pair(1, kt // 2, nc.gpsimd)
            quad = psA.tile([128, 4, 512], f32, tag="quad", name=f"q1_{kt}")
            for ci in range(NB1):
                lhs = w2blk[1][ci][:, kt % 2, :]
                for sj in range(4):
                    o = quad[:, sj, :].rearrange("p (s l) -> p s l", s=2)
                    nc.tensor.matmul(
                        out=o[:, :, 0:L1],
                        lhsT=lhs,
                        rhs=x1sb[ci][:, 2 * sj:2 * sj + 2, :],
                        start=(ci == 0),
                        stop=(ci == NB1 - 1),
                    )
            v = quad[:].rearrange("p q (s l) -> p q s l", s=2)[:, :, :, 0:L1]
            w1b = w1sb1[kt][:].rearrange("p (a b l) -> p a b l", a=1, b=1) \
                .to_broadcast([128, 4, 2, L1])
            nc.vector.tensor_mul(v, v, w1b)
            nc.vector.tensor_reduce(
                out=h1sb[:, kt * 8:(kt + 1) * 8], in_=v, axis=X, op=ALU.add)

        psA_ctx.__exit__(None, None, None)
        ctx_brx.__exit__(None, None, None)

        # remaining embed weight tiles into the space freed by the x pools;
        # the DMAs wait on the last stage-A reads, then flow during the
        # AllGather / BN / stage-C / AllReduce gap.
        ewpB = stack.enter_context(tc.tile_pool(name="ewpB", bufs=1))
        for pc in range(EWA, EWA + EWB):
            t = ewpB.tile([128, W], bf16, tag=f"ewB{pc}", name=f"ewB{pc}")
            nc.scalar.dma_start(out=t, in_=ewt[pc, :, :])
            ewsb[pc] = t

        # ---------------- stage B : AllGather h + BatchNorm + latent ----------------
        nc.sync.dma_start(out=hb_local[1], in_=h1sb[:, :])
        hg = dpool.tile([NCORES, 2, 128, N], f32, addr_space="Shared")
        nc.gpsimd.collective_compute(
            "AllGather",
            ALU.bypass,
            replica_groups=[list(range(NCORES))],
            ins=[hb_local[:].opt()],
            outs=[hg[:].opt()],
        )

        # load gathered h split across both rings: hall[p, g, b, col]
        GH = NCORES // 2
        nc.sync.dma_start(
            out=hall[:, 0:GH].rearrange("p g b n -> p (g b) n"),
            in_=hg[0:GH].rearrange("g b p n -> p (g b) n"))
        nc.gpsimd.dma_start(
            out=hall[:, GH:].rearrange("p g b n -> p (g b) n"),
            in_=hg[GH:].rearrange("g b p n -> p (g b) n"))
        touch(hall[:, 0, 0, :])
        touch(hall[:, GH, 0, :])

        stx = stack.enter_context(tc.tile_pool(name="stats", bufs=2))
        hsq = singles.tile([128, NCORES, 2, N], f32)
        s1 = singles.tile([128, NCORES, 2, KT], f32)
        s1q = singles.tile([128, NCORES, 2, KT], f32)
        sums = singles.tile([128, 2, KT], f32)     # becomes mean
        sumq = singles.tile([128, 2, KT], f32)     # becomes var
        tmp_ = singles.tile([128, 2, KT], f32)
        nc.vector.tensor_mul(hsq[:].rearrange("p g b n -> p (g b n)"),
                             hall[:].rearrange("p g b n -> p (g b n)"),
                             hall[:].rearrange("p g b n -> p (g b n)"))
        # reduce over samples within each (g, b, kt), then over g
        nc.vector.tensor_reduce(
            out=s1[:].rearrange("p g b k -> p (g b k)"),
            in_=hall[:].rearrange("p g b (k n) -> p (g b k) n", k=KT),
            axis=X, op=ALU.add)
        nc.vector.tensor_reduce(
            out=s1q[:].rearrange("p g b k -> p (g b k)"),
            in_=hsq[:].rearrange("p g b (k n) -> p (g b k) n", k=KT),
            axis=X, op=ALU.add)
        nc.vector.tensor_reduce(
            out=sums[:].rearrange("p b k -> p (b k)"),
            in_=s1[:].rearrange("p g b k -> p (b k) g"),
            axis=X, op=ALU.add)
        nc.vector.tensor_reduce(
            out=sumq[:].rearrange("p b k -> p (b k)"),
            in_=s1q[:].rearrange("p g b k -> p (b k) g"),
            axis=X, op=ALU.add)
        # mean/var (biased); rstd = 1/sqrt(var+eps); a = 0.5*gamma*rstd;
        # c = 0.5*beta - mu*a
        nc.scalar.activation(out=sums[:], in_=sums[:], func=ACTF.Copy,
                             scale=1.0 / N)
        nc.scalar.activation(out=sumq[:], in_=sumq[:], func=ACTF.Copy,
                             scale=1.0 / N)
        nc.vector.tensor_mul(tmp_[:], sums[:], sums[:])
        nc.vector.tensor_sub(sumq[:], sumq[:], tmp_[:])
        a_ = singles.tile([128, 2, KT], f32)
        c_ = singles.tile([128, 2, KT], f32)
        nc.scalar.activation(out=a_[:], in_=sumq[:], func=ACTF.Sqrt,
                             bias=epssb, scale=1.0)
        nc.vector.reciprocal(out=a_[:].rearrange("p b k -> p (b k)"),
                             in_=a_[:].rearrange("p b k -> p (b k)"))
        nc.vector.tensor_mul(a_[:], a_[:], gbsb[:])
        nc.vector.tensor_mul(c_[:], sums[:], a_[:])
        nc.vector.tensor_sub(c_[:], btsb[:], c_[:])

        # latent = h0*a0 + h1*a1 + (c0+c1), then lm[k, g*8+n] = latent*mask
        lat4 = singles.tile([128, NCORES, KT, NS], f32)
        scr4 = singles.tile([128, NCORES, KT, NS], f32)
        cb = singles.tile([128, KT], f32)

        def _bk4(ap):
            return ap.rearrange("p (a k b) -> p a k b", a=1, b=1) \
                .to_broadcast([128, NCORES, KT, NS])

        h0v = hall[:, :, 0, :].rearrange("p g (k n) -> p g k n", k=KT)
        h1v = hall[:, :, 1, :].rearrange("p g (k n) -> p g k n", k=KT)
        nc.vector.tensor_mul(lat4[:], h0v, _bk4(a_[:, 0, :]))
        nc.vector.tensor_mul(scr4[:], h1v, _bk4(a_[:, 1, :]))
        lat4f = lat4[:].rearrange("p g k n -> p (g k n)")
        nc.vector.tensor_add(lat4f, lat4f, scr4[:].rearrange("p g k n -> p (g k n)"))
        nc.vector.tensor_add(cb[:], c_[:, 0, :], c_[:, 1, :])
        nc.vector.tensor_add(lat4[:], lat4[:], _bk4(cb[:]))
        maskb = masksb[:].rearrange("p (o g n) -> p o g n", o=1, g=NCORES) \
            .to_broadcast([128, KT, NCORES, NS])
        nc.vector.tensor_mul(
            lm_all[:].rearrange("p k (g n) -> p k g n", g=NCORES),
            lat4[:].rearrange("p g k n -> p k g n"),
            maskb)

        # ---------------- stage C : z partial = cwt.T @ lm + fcb x mask ----------------
        zfc_local = dpool.tile([2, 128, 8 * N], bf16)
        with tc.tile_pool(name="zps", bufs=1, space="PSUM") as zps:
            zp = []
            for half in range(2):
                t = zps.tile([128, 8 * N], f32, tag=f"zp{half}", name=f"zp{half}")
                zp.append(t)
            for pt in range(PT):
                o = zp[pt // 8][:, (pt % 8) * N:(pt % 8 + 1) * N]
                for kt in range(KT):
                    nc.tensor.matmul(
                        out=o,
                        lhsT=cwsb[kt][:, pt * 128:(pt + 1) * 128],
                        rhs=lm_all[:, kt, :],
                        start=(kt == 0), stop=False,
                    )
                nc.tensor.matmul(
                    out=o, lhsT=fcbsb[:, pt * 128:(pt + 1) * 128],
                    rhs=maskrsb[:, :], start=False, stop=True)
            for half in range(2):
                zhb = stx.tile([128, 8 * N], bf16, tag=f"zh{half}", name=f"zh{half}")
                nc.vector.tensor_copy(out=zhb, in_=zp[half][:, :])
                nc.sync.dma_start(out=zfc_local[half], in_=zhb)

        zr = dpool.tile([2, 128, 8 * N], bf16, addr_space="Shared")
        nc.gpsimd.collective_compute(
            "AllReduce",
            ALU.add,
            replica_groups=[list(range(NCORES))],
            ins=[zfc_local[:].opt()],
            outs=[zr[:].opt()],
        )
        nc.sync.dma_start(
            out=zsb_all[:],
            in_=zr[:].rearrange("h p c -> p h c"))
        touch(zsb_all[:, 0, :])

        def zview(pc):
            return zsb_all[:, pc // 8, (pc % 8) * N:(pc % 8 + 1) * N]

        # ---------------- stage D : out = z.T @ ewt + 2*eb ----------------
        with tc.tile_pool(name="ewpC", bufs=2) as ewpC, \
             tc.tile_pool(name="odp", bufs=2, space="PSUM") as odp, \
             tc.tile_pool(name="osp", bufs=3) as osp:
            for nb in range(NBLK):
                bs = nb * 512
                bw = min(512, W - bs)
                echunk = {}
                for k, pc in enumerate(range(EWA + EWB, PT)):
                    t = ewpC.tile([128, 512], bf16, tag=f"s{pc}",
                                  name=f"ewC{nb}_{pc}")
                    eng = nc.sync if k % 2 == 0 else nc.gpsimd
                    eng.dma_start(out=t[:, :bw], in_=ewt[pc, :, bs:bs + bw])
                    echunk[pc] = t
                odA = odp.tile([128, 512], f32, tag="odA", name=f"odA{nb}")
                odB = odp.tile([128, 512], f32, tag="odB", name=f"odB{nb}")

                def esrc(pc):
                    if ewsb[pc] is not None:
                        return ewsb[pc][:, bs:bs + bw]
                    return echunk[pc][:, :bw]

                # bias into the lower half's accumulator
                nc.tensor.matmul(
                    out=odA[0:64, :bw], lhsT=ones1[:, :], rhs=eb2sb[:, bs:bs + bw],
                    start=True, stop=False, tile_position=(0, 0))
                for pc in range(8):
                    nc.tensor.matmul(
                        out=odA[0:64, :bw], lhsT=zview(pc), rhs=esrc(pc),
                        start=False, stop=(pc == 7), tile_position=(0, 0))
                    nc.tensor.matmul(
                        out=odB[64:128, :bw], lhsT=zview(pc + 8), rhs=esrc(pc + 8),
                        start=(pc == 0), stop=(pc == 7), tile_position=(0, 64))
                osb = osp.tile([64, 512], f32, tag="osb", name=f"osb{nb}")
                nc.scalar.activation(out=osb[:, :bw], in_=odA[0:64, :bw],
                                     func=ACTF.Copy)
                nc.vector.tensor_add(osb[:, :bw], osb[:, :bw], odB[64:128, :bw])
                nc.scalar.dma_start(out=out[:, bs:bs + bw], in_=osb[:, :bw])

    nc.compile()
    return nc


def _host_prep(x0, x1, w1_0, w2_0, gamma0, beta0, w1_1, w2_1, gamma1, beta1,
               shared_w, fc_w, fc_b, embed_w, embed_b, indices):
    import ml_dtypes
    f = np.float32
    bf = ml_dtypes.bfloat16
    x0t = np.ascontiguousarray(x0.transpose(2, 0, 1)).astype(bf)   # [1024, 64, 257]
    x1t = np.ascontiguousarray(x1.transpose(2, 0, 1)).astype(bf)   # [768, 64, 197]
    # w2 transposed, columns grouped in kt-pairs: [KT//2, C, 256]
    w2_0t = w2_0.T.astype(bf)                                      # [C0, H]
    w2_1t = w2_1.T.astype(bf)
    w2_0pp = np.ascontiguousarray(
        w2_0t.reshape(C0, KT // 2, 256).transpose(1, 0, 2))
    w2_1pp = np.ascontiguousarray(
        w2_1t.reshape(C1, KT // 2, 256).transpose(1, 0, 2))
    gbp = np.empty((128, 2, KT), f)
    btp = np.empty((128, 2, KT), f)
    gbp[:, 0, :] = (gamma0 * 0.5).reshape(KT, 128).T
    gbp[:, 1, :] = (gamma1 * 0.5).reshape(KT, 128).T
    btp[:, 0, :] = (beta0 * 0.5).reshape(KT, 128).T
    btp[:, 1, :] = (beta1 * 0.5).reshape(KT, 128).T
    swt = shared_w.T.astype(f)                                    # [1024, 2048]
    fcwt = fc_w.T.astype(f)                                       # [1024, 16384]
    ewt_pad = np.zeros((P, NCORES * W), dtype=bf)
    ewt_pad[:, :KE] = embed_w.T.astype(bf)
    eb2_pad = np.zeros((1, NCORES * W), dtype=bf)
    eb2_pad[0, :KE] = (2.0 * embed_b).astype(bf)

    idx = np.asarray(indices).astype(np.int64)
    in_maps = []
    for i in range(NCORES):
        m = (idx == i).astype(f)
        ew_core = ewt_pad[:, i * W:(i + 1) * W]                   # [2048, W]
        in_maps.append({
            "x0t": np.ascontiguousarray(x0t[:, i * NS:(i + 1) * NS, :]),
            "x1t": np.ascontiguousarray(x1t[:, i * NS:(i + 1) * NS, :]),
            "w2_0p": w2_0pp,
            "w2_1p": w2_1pp,
            "w1_0": np.ascontiguousarray(w1_0, dtype=f),
            "w1_1": np.ascontiguousarray(w1_1, dtype=f),
            "gb": gbp, "bt": btp,
            "cwt": np.ascontiguousarray(swt + fcwt[:, i * P:(i + 1) * P]).astype(bf),
            "fcb": np.ascontiguousarray(fc_b[i * P:(i + 1) * P].reshape(1, P)).astype(bf),
            "maskrow": np.ascontiguousarray(m.reshape(1, N)).astype(bf),
            "mask": np.ascontiguousarray(np.broadcast_to(m, (128, N))),
            "ewt": np.ascontiguousarray(
                ew_core.reshape(PT, 128, W)),
            "eb2": np.ascontiguousarray(eb2_pad[:, i * W:(i + 1) * W]),
        })
    return in_maps


def kernel(**inputs):
    if "/opt/trn_rl_repo" not in sys.path:
        sys.path.insert(0, "/opt/trn_rl_repo")
    from concourse.bass_utils import run_bass_kernel_spmd

    in_maps = _host_prep(**inputs)
    if "nc" not in _CACHE:
        _CACHE["nc"] = _build_nc()
    nc = _CACHE["nc"]
    res = run_bass_kernel_spmd(nc, in_maps, core_ids=list(range(NCORES)))
    outs = [np.asarray(res.results[i]["out"]) for i in range(NCORES)]
    full = np.concatenate(outs, axis=1)[:, :KE]
    return np.ascontiguousarray(full, dtype=np.float32)


if __name__ == "__main__":
    sys.path.insert(0, os.path.dirname(os.path.abspath(__file__)))
    import reference
    inputs = {k: np.asarray(v) for k, v in reference.setup_inputs().items()}
    expected = np.asarray(reference.reference(**inputs))
    actual = kernel(**inputs)
    err = np.abs(actual - expected).max() / (np.abs(expected).max() + 1e-12)
    print("Relative error:", err)


# revision 28
# speedup vs baseline: 1.7808x; 1.0340x over previous
"""Trainium2 Bass kernel for nn_GroupLinearEncoder.

Math (reference):
  h_b = feat_proj(x_b) = BN(einsum over l,c of x_b and w1_b, w2_b)   (N,1024)
  latent = 0.5*(bn(h0)+bn(h1))
  out = (latent @ shared_w.T + subj) @ embed_w.T + 2*embed_b
  where subj = einsum(latent, fc_w[indices]) + b_sel.

Key algebraic folds:
  * group_pred + subj_res share the embed matmul: z = latent@shared_w.T + subj,
    out = z @ embed_w.T + 2*embed_b  -> embed_w is read ONCE.
  * Because every sample belongs to exactly one group, per-core
    cwt_i = shared_w.T + fc_w.T[:, group_i] applied to mask-selected samples
    and AllReduced over cores yields z directly (shared term included).

Sharding over 8 cores:
  * feat_proj: data-parallel over batch (8 samples/core), AllGather h.
  * z: group-parallel (core i handles group i via sample masks), AllReduce (bf16).
  * embed: column-parallel over out_dim (4944 rows/core, padded), concat on host.

Performance structure (vs v1):
  * embed_w (20.3MB/core) is mostly SBUF-resident: 6 p-tiles prefetched during
    stage A, 6 more during the AllGather/BN/C/AllReduce gap (into SBUF freed by
    the x pools), 4 streamed per 512-col block during stage D.
  * stage D matmuls packed 2x via PE column-group tiling (out partitions 0:64
    and 64:128 run concurrently), recovering the M=64 half-array loss.
  * stage A DVE work batched: one mul+reduce per 4-sample PSUM quad tile with
    stride-0 broadcast w1, instead of per-sample ops.
  * BN stats batched per (branch, ktile) via one bn_stats/bn_aggr pair; BN
    affine applied on the Scalar engine (activation scale/bias APs).
  * z AllReduce in bf16 (exact: per column only one core contributes nonzeros).
"""

import os
import sys

import numpy as np

N, H, P, KE = 64, 1024, 2048, 39548
NCORES = 8
NS = N // NCORES            # samples per core
L0, C0 = 257, 1024
L1, C1 = 197, 768
W = 4944                    # embed rows per core (8*4944 = 39552, 4 pad)
PT = P // 128               # 16
KT = H // 128               # 8
NB0 = C0 // 128             # 8
NB1 = C1 // 128             # 6
BN_EPS = 1e-5
EWA = 5                     # ewt p-tiles resident from stage A
EWB = 6                     # ewt p-tiles loaded during the collective gap
NBLK = (W + 511) // 512     # 10 output blocks

_CACHE = {}


def _build_nc():
    if "/opt/trn_rl_repo" not in sys.path:
        sys.path.insert(0, "/opt/trn_rl_repo")
    import concourse.bass as bass
    import concourse.tile as tile
    from concourse import bacc, mybir
    from contextlib import ExitStack

    f32 = mybir.dt.float32
    bf16 = mybir.dt.bfloat16
    ALU = mybir.AluOpType
    ACTF = mybir.ActivationFunctionType
    X = mybir.AxisListType.X

    nc = bacc.Bacc(num_devices=NCORES)

    x0t = nc.declare_dram_parameter("x0t", [C0, NS, L0], bf16, isOutput=False)
    x1t = nc.declare_dram_parameter("x1t", [C1, NS, L1], bf16, isOutput=False)
    # w2 transposed + kt-paired: [KT//2, C, 256] so each DMA line is 512B
    w2_0p = nc.declare_dram_parameter("w2_0p", [KT // 2, C0, 256], bf16, isOutput=False)
    w2_1p = nc.declare_dram_parameter("w2_1p", [KT // 2, C1, 256], bf16, isOutput=False)
    w1_0 = nc.declare_dram_parameter("w1_0", [H, L0], f32, isOutput=False)
    w1_1 = nc.declare_dram_parameter("w1_1", [H, L1], f32, isOutput=False)
    gb = nc.declare_dram_parameter("gb", [128, 2, KT], f32, isOutput=False)
    bt = nc.declare_dram_parameter("bt", [128, 2, KT], f32, isOutput=False)
    cwt = nc.declare_dram_parameter("cwt", [H, P], bf16, isOutput=False)
    fcb = nc.declare_dram_parameter("fcb", [1, P], bf16, isOutput=False)
    maskrow = nc.declare_dram_parameter("maskrow", [1, N], bf16, isOutput=False)
    mask = nc.declare_dram_parameter("mask", [128, N], f32, isOutput=False)
    ewt = nc.declare_dram_parameter("ewt", [PT, 128, W], bf16, isOutput=False)
    eb2 = nc.declare_dram_parameter("eb2", [1, W], bf16, isOutput=False)
    out = nc.declare_dram_parameter("out", [N, W], f32, isOutput=True)

    with tile.TileContext(nc) as tc, ExitStack() as stack:
        singles = stack.enter_context(tc.tile_pool(name="singles", bufs=1))
        dpool = stack.enter_context(tc.tile_pool(name="dram", bufs=1, space="DRAM"))
        tpool = stack.enter_context(tc.tile_pool(name="touchp", bufs=2))
        _tn = [0]

        def touch(ap):
            # absorb a DMA's queue semaphores into DVE's vector clock so
            # downstream DVE ops need only engine-local ordering
            _tn[0] += 1
            tt = tpool.tile([ap.shape[0], 1], ap.dtype, tag="touch",
                            name=f"touch{_tn[0]}")
            nc.vector.tensor_copy(out=tt, in_=ap[:, 0:1])

        # --- persistent small tensors ---
        h0sb = singles.tile([128, N], f32)       # col = kt*8 + n_local
        h1sb = singles.tile([128, N], f32)
        gbsb = singles.tile([128, 2, KT], f32)   # 0.5*gamma, [*, branch, kt]
        btsb = singles.tile([128, 2, KT], f32)   # 0.5*beta
        masksb = singles.tile([128, N], f32)
        epssb = singles.tile([128, 1], f32)
        hall = singles.tile([128, NCORES, 2, N], bf16)  # gathered h
        lm_all = singles.tile([128, KT, N], bf16)  # latent*mask per ktile
        zsb_all = singles.tile([128, 2, 8 * N], bf16)    # z after AllReduce
        fcbsb = singles.tile([1, P], bf16)
        maskrsb = singles.tile([1, N], bf16)
        eb2sb = singles.tile([1, W], bf16)
        ones1 = singles.tile([1, N], bf16)
        gate = singles.tile([128, 1], f32)

        cwtp = stack.enter_context(tc.tile_pool(name="cwtp", bufs=1))
        ewpA = stack.enter_context(tc.tile_pool(name="ewpA", bufs=1))
        # streamed stage-D chunks live in fresh SBUF (no WAR on the stage-A
        # pools) so their first two blocks load during startup, not at the
        # stage-A -> AllGather boundary where they would delay the h stores
        ewpC = stack.enter_context(tc.tile_pool(name="ewpC", bufs=2))

        # ---- stage-A input pool (freed after stage A; space reused by ewpB)
        ctx_brx = tc.tile_pool(name="brx", bufs=1)
        brx = ctx_brx.__enter__()

        # DMAs on one HWDGE ring execute FIFO (~175GB/s per ring), so the
        # startup-critical loads are split across the sync and gpsimd rings:
        #   sync:   w2_0 pair 0, x0 even ci  (first-matmul dependencies)
        #   gpsimd: x0 odd ci, x1, w1, small singles
        x0sb = [None] * NB0
        x1sb = []
        w2blk = {}

        def _w2_tile(branch, ci, kp):
            return brx.tile([128, 2, 128], bf16, tag=f"w2_{branch}_{ci}", bufs=2,
                            name=f"w2_{branch}_{kp}_{ci}")

        def _load_w2_pair(branch, kp, eng):
            src = w2_0p if branch == 0 else w2_1p
            nb = NB0 if branch == 0 else NB1
            blks = []
            for ci in range(nb):
                t = _w2_tile(branch, ci, kp)
                eng.dma_start(
                    out=t, in_=src[kp, ci * 128:(ci + 1) * 128, :].rearrange(
                        "c (j u) -> c j u", j=2))
                blks.append(t)
            return blks

        w2blk[0] = _load_w2_pair(0, 0, nc.sync)
        for ci in range(NB0):
            t = brx.tile([128, NS, L0], bf16, tag=f"x0_{ci}", name=f"x0_{ci}")
            eng = nc.sync if ci % 2 == 0 else nc.gpsimd
            eng.dma_start(out=t, in_=x0t[ci * 128:(ci + 1) * 128, :, :])
            x0sb[ci] = t
        for ci in range(NB1):
            t = brx.tile([128, NS, L1], bf16, tag=f"x1_{ci}", name=f"x1_{ci}")
            nc.gpsimd.dma_start(out=t, in_=x1t[ci * 128:(ci + 1) * 128, :, :])
            x1sb.append(t)
        w1sb0, w1sb1 = [], []
        for kt in range(KT):
            t = brx.tile([128, L0], f32, tag=f"w10_{kt}", name=f"w10_{kt}")
            nc.gpsimd.dma_start(out=t, in_=w1_0[kt * 128:(kt + 1) * 128, :])
            touch(t)
            w1sb0.append(t)
        for kt in range(KT):
            t = brx.tile([128, L1], f32, tag=f"w11_{kt}", name=f"w11_{kt}")
            nc.gpsimd.dma_start(out=t, in_=w1_1[kt * 128:(kt + 1) * 128, :])
            touch(t)
            w1sb1.append(t)

        nc.gpsimd.dma_start(out=gbsb, in_=gb[:, :, :])
        nc.gpsimd.dma_start(out=btsb, in_=bt[:, :, :])
        nc.gpsimd.dma_start(out=masksb, in_=mask[:, :])
        nc.gpsimd.dma_start(out=fcbsb, in_=fcb[:, :])
        nc.gpsimd.dma_start(out=maskrsb, in_=maskrow[:, :])
        touch(gbsb[:].rearrange("p a b -> p (a b)"))
        touch(btsb[:].rearrange("p a b -> p (a b)"))
        touch(masksb)
        nc.vector.memset(epssb, BN_EPS)
        nc.vector.memset(ones1, 1.0)

        psA_ctx = tc.tile_pool(name="psA", bufs=2, space="PSUM")
        psA = psA_ctx.__enter__()

        # ---------------- stage A : feat_proj ----------------
        cwsb = []
        ewsb = [None] * PT

        # branch 0: 4-sample PSUM quads; one DVE mul+reduce per quad
        for kt in range(KT):
            if kt % 2 == 0 and kt > 0:
                w2blk[0] = _load_w2_pair(0, kt // 2, nc.sync)
            for grp in range(2):
                quad = psA.tile([128, 4, 512], f32, tag="quad",
                                name=f"q0_{kt}_{grp}")
                for ci in range(NB0):
                    lhs = w2blk[0][ci][:, kt % 2, :]
                    for j in range(4):
                        n = grp * 4 + j
                        nc.tensor.matmul(
                            out=quad[:, j, 0:L0],
                            lhsT=lhs,
                            rhs=x0sb[ci][:, n, :],
                            start=(ci == 0),
                            stop=(ci == NB0 - 1),
                        )
                w1b = w1sb0[kt][:].rearrange("p (o l) -> p o l", o=1) \
                    .to_broadcast([128, 4, L0])
                nc.vector.tensor_mul(quad[:, :, 0:L0], quad[:, :, 0:L0], w1b)
                c0 = kt * 8 + grp * 4
                nc.vector.tensor_reduce(
                    out=h0sb[:, c0:c0 + 4], in_=quad[:, :, 0:L0],
                    axis=X, op=ALU.add)
            if kt == 2:
                # gate bulk scalar-queue loads behind early stage-A output so
                # x/w2 win HBM bandwidth during startup
                nc.scalar.activation(out=gate, in_=h0sb[:, 23:24], func=ACTF.Copy)
                for kt2 in range(KT):
                    t = cwtp.tile([128, P], bf16, tag=f"cw{kt2}", name=f"cw{kt2}")
                    nc.scalar.dma_start(out=t, in_=cwt[kt2 * 128:(kt2 + 1) * 128, :])
                    cwsb.append(t)
                for pc in range(EWA):
                    t = ewpA.tile([128, W], bf16, tag=f"ewA{pc}", name=f"ewA{pc}")
                    nc.scalar.dma_start(out=t, in_=ewt[pc, :, :])
                    ewsb[pc] = t
                nc.scalar.dma_start(out=eb2sb, in_=eb2[:, :])

        # h0 store early so the AllGather input is half-ready before br1 ends;
        # bf16 cast on DVE, then a plain store on the gpsimd ring (idle at the
        # end of stage A, so the store completes promptly)
        hb_local = dpool.tile([2, 128, N], bf16)
        w2blk[1] = _load_w2_pair(1, 0, nc.gpsimd)
        h0b = singles.tile([128, N], bf16)
        h1b = singles.tile([128, N], bf16)
        nc.vector.tensor_copy(out=h0b, in_=h0sb[:, :])
        nc.gpsimd.dma_start(out=hb_local[0], in_=h0b)

        # branch 1: two samples per matmul; one DVE mul+reduce per kt
        for kt in range(KT):
            if kt % 2 == 0 and kt > 0:
                w2blk[1] = _load_w2_pair(1, kt // 2, nc.gpsimd)
            quad = psA.tile([128, 4, 512], f32, tag="quad", name=f"q1_{kt}")
            for ci in range(NB1):
                lhs = w2blk[1][ci][:, kt % 2, :]
                for sj in range(4):
                    o = quad[:, sj, :].rearrange("p (s l) -> p s l", s=2)
                    nc.tensor.matmul(
                        out=o[:, :, 0:L1],
                        lhsT=lhs,
                        rhs=x1sb[ci][:, 2 * sj:2 * sj + 2, :],
                        start=(ci == 0),
                        stop=(ci == NB1 - 1),
                    )
            v = quad[:].rearrange("p q (s l) -> p q s l", s=2)[:, :, :, 0:L1]
            w1b = w1sb1[kt][:].rearrange("p (a b l) -> p a b l", a=1, b=1) \
                .to_broadcast([128, 4, 2, L1])
            nc.vector.tensor_mul(v, v, w1b)
            nc.vector.tensor_reduce(
                out=h1sb[:, kt * 8:(kt + 1) * 8], in_=v, axis=X, op=ALU.add)

        psA_ctx.__exit__(None, None, None)
        ctx_brx.__exit__(None, None, None)

        # remaining embed weight tiles into the space freed by the x pools;
        # the DMAs wait on the last stage-A reads, then flow during the
        # AllGather / BN / stage-C / AllReduce gap.
        ewpB = stack.enter_context(tc.tile_pool(name="ewpB", bufs=1))
        for pc in range(EWA, EWA + EWB):
            t = ewpB.tile([128, W], bf16, tag=f"ewB{pc}", name=f"ewB{pc}")
            nc.scalar.dma_start(out=t, in_=ewt[pc, :, :])
            ewsb[pc] = t

        # ---------------- stage B : AllGather h + BatchNorm + latent ----------------
        nc.vector.tensor_copy(out=h1b, in_=h1sb[:, :])
        nc.gpsimd.dma_start(out=hb_local[1], in_=h1b)
        hg = dpool.tile([NCORES, 2, 128, N], bf16, addr_space="Shared")
        nc.gpsimd.collective_compute(
            "AllGather",
            ALU.bypass,
            replica_groups=[list(range(NCORES))],
            ins=[hb_local[:].opt()],
            outs=[hg[:].opt()],
        )

        # load gathered h split across both rings: hall[p, g, b, col]
        GH = NCORES // 2
        nc.sync.dma_start(
            out=hall[:, 0:GH].rearrange("p g b n -> p (g b) n"),
            in_=hg[0:GH].rearrange("g b p n -> p (g b) n"))
        nc.gpsimd.dma_start(
            out=hall[:, GH:].rearrange("p g b n -> p (g b) n"),
            in_=hg[GH:].rearrange("g b p n -> p (g b) n"))
        touch(hall[:, 0, 0, :])
        touch(hall[:, GH, 0, :])

        stx = stack.enter_context(tc.tile_pool(name="stats", bufs=2))
        hsq = singles.tile([128, NCORES, 2, N], f32)
        s1 = singles.tile([128, NCORES, 2, KT], f32)
        s1q = singles.tile([128, NCORES, 2, KT], f32)
        sums = singles.tile([128, 2, KT], f32)     # becomes mean
        sumq = singles.tile([128, 2, KT], f32)     # becomes var
        tmp_ = singles.tile([128, 2, KT], f32)
        nc.vector.tensor_mul(hsq[:].rearrange("p g b n -> p (g b n)"),
                             hall[:].rearrange("p g b n -> p (g b n)"),
                             hall[:].rearrange("p g b n -> p (g b n)"))
        # reduce over samples within each (g, b, kt), then over g
        nc.vector.tensor_reduce(
            out=s1[:].rearrange("p g b k -> p (g b k)"),
            in_=hall[:].rearrange("p g b (k n) -> p (g b k) n", k=KT),
            axis=X, op=ALU.add)
        nc.vector.tensor_reduce(
            out=s1q[:].rearrange("p g b k -> p (g b k)"),
            in_=hsq[:].rearrange("p g b (k n) -> p (g b k) n", k=KT),
            axis=X, op=ALU.add)
        nc.vector.tensor_reduce(
            out=sums[:].rearrange("p b k -> p (b k)"),
            in_=s1[:].rearrange("p g b k -> p (b k) g"),
            axis=X, op=ALU.add)
        nc.vector.tensor_reduce(
            out=sumq[:].rearrange("p b k -> p (b k)"),
            in_=s1q[:].rearrange("p g b k -> p (b k) g"),
            axis=X, op=ALU.add)
        # mean/var (biased); rstd = 1/sqrt(var+eps); a = 0.5*gamma*rstd;
        # c = 0.5*beta - mu*a
        nc.vector.tensor_scalar_mul(sums[:], sums[:], 1.0 / N)
        nc.vector.tensor_scalar_mul(sumq[:], sumq[:], 1.0 / N)
        nc.vector.tensor_mul(tmp_[:], sums[:], sums[:])
        nc.vector.tensor_sub(sumq[:], sumq[:], tmp_[:])
        a_ = singles.tile([128, 2, KT], f32)
        c_ = singles.tile([128, 2, KT], f32)
        nc.scalar.activation(out=a_[:], in_=sumq[:], func=ACTF.Sqrt,
                             bias=epssb, scale=1.0)
        nc.vector.reciprocal(out=a_[:].rearrange("p b k -> p (b k)"),
                             in_=a_[:].rearrange("p b k -> p (b k)"))
        nc.vector.tensor_mul(a_[:], a_[:], gbsb[:])
        nc.vector.tensor_mul(c_[:], sums[:], a_[:])
        nc.vector.tensor_sub(c_[:], btsb[:], c_[:])

        # latent = h0*a0 + h1*a1 + (c0+c1), then lm[k, g*8+n] = latent*mask
        lat4 = singles.tile([128, NCORES, KT, NS], f32)
        scr4 = singles.tile([128, NCORES, KT, NS], f32)
        cb = singles.tile([128, KT], f32)

        def _bk4(ap):
            return ap.rearrange("p (a k b) -> p a k b", a=1, b=1) \
                .to_broadcast([128, NCORES, KT, NS])

        nc.vector.tensor_add(cb[:], c_[:, 0, :], c_[:, 1, :])
        KH = KT // 2
        maskb = masksb[:].rearrange("p (o g n) -> p o g n", o=1, g=NCORES) \
            .to_broadcast([128, KH, NCORES, NS])
        for hh in range(2):
            ks = hh * KH

            def _bk4(ap):
                return ap.rearrange("p (a k b) -> p a k b", a=1, b=1) \
                    .to_broadcast([128, NCORES, KH, NS])

            h0v = hall[:, :, 0, ks * 8:(ks + KH) * 8] \
                .rearrange("p g (k n) -> p g k n", k=KH)
            h1v = hall[:, :, 1, ks * 8:(ks + KH) * 8] \
                .rearrange("p g (k n) -> p g k n", k=KH)
            l4 = lat4[:, :, ks:ks + KH, :]
            s4 = scr4[:, :, ks:ks + KH, :]
            nc.vector.tensor_mul(l4, h0v, _bk4(a_[:, 0, ks:ks + KH]))
            nc.vector.tensor_mul(s4, h1v, _bk4(a_[:, 1, ks:ks + KH]))
            nc.vector.tensor_add(l4, l4, s4)
            nc.vector.tensor_add(l4, l4, _bk4(cb[:, ks:ks + KH]))
            nc.vector.tensor_mul(
                lm_all[:, ks:ks + KH, :].rearrange("p k (g n) -> p k g n",
                                                   g=NCORES),
                lat4[:, :, ks:ks + KH, :].rearrange("p g k n -> p k g n"),
                maskb)

        # ---------------- stage C : z partial = cwt.T @ lm + fcb x mask ----------------
        zfc_local = dpool.tile([2, 128, 8 * N], bf16)
        with tc.tile_pool(name="zps", bufs=1, space="PSUM") as zps:
            zp = []
            for half in range(2):
                t = zps.tile([128, 8 * N], f32, tag=f"zp{half}", name=f"zp{half}")
                zp.append(t)

            def zpo(pt):
                return zp[pt // 8][:, (pt % 8) * N:(pt % 8 + 1) * N]

            # start=True clears has_written for the WHOLE bank, so only the
            # first matmul touching each bank may set it (kt 0, pt 0/8);
            # later flags=0 matmuls overwrite-where-unset / accumulate-where-set
            for kt in range(KT):
                for pt in range(PT):
                    nc.tensor.matmul(
                        out=zpo(pt),
                        lhsT=cwsb[kt][:, pt * 128:(pt + 1) * 128],
                        rhs=lm_all[:, kt, :],
                        start=(kt == 0 and pt % 8 == 0), stop=False,
                        skip_group_check=True,
                    )
            for pt in range(PT):
                nc.tensor.matmul(
                    out=zpo(pt), lhsT=fcbsb[:, pt * 128:(pt + 1) * 128],
                    rhs=maskrsb[:, :], start=False, stop=True)
            for half in range(2):
                zhb = stx.tile([128, 8 * N], bf16, tag=f"zh{half}", name=f"zh{half}")
                nc.vector.tensor_copy(out=zhb, in_=zp[half][:, :])
                nc.sync.dma_start(out=zfc_local[half], in_=zhb)

        zr = dpool.tile([2, 128, 8 * N], bf16, addr_space="Shared")
        nc.gpsimd.collective_compute(
            "AllReduce",
            ALU.add,
            replica_groups=[list(range(NCORES))],
            ins=[zfc_local[:].opt()],
            outs=[zr[:].opt()],
        )
        nc.sync.dma_start(
            out=zsb_all[:],
            in_=zr[:].rearrange("h p c -> p h c"))
        touch(zsb_all[:, 0, :])

        def zview(pc):
            return zsb_all[:, pc // 8, (pc % 8) * N:(pc % 8 + 1) * N]

        # ---------------- stage D : out = z.T @ ewt + 2*eb ----------------
        with tc.tile_pool(name="odp", bufs=2, space="PSUM") as odp, \
             tc.tile_pool(name="osp", bufs=3) as osp:
            for nb in range(NBLK):
                bs = nb * 512
                bw = min(512, W - bs)
                echunk = {}
                for k, pc in enumerate(range(EWA + EWB, PT)):
                    t = ewpC.tile([128, 512], bf16, tag=f"s{pc}",
                                  name=f"ewC{nb}_{pc}")
                    eng = nc.sync if k % 2 == 0 else nc.gpsimd
                    eng.dma_start(out=t[:, :bw], in_=ewt[pc, :, bs:bs + bw])
                    echunk[pc] = t
                odA = odp.tile([128, 512], f32, tag="odA", name=f"odA{nb}")
                odB = odp.tile([128, 512], f32, tag="odB", name=f"odB{nb}")

                def esrc(pc):
                    if ewsb[pc] is not None:
                        return ewsb[pc][:, bs:bs + bw]
                    return echunk[pc][:, :bw]

                # bias into the lower half's accumulator
                nc.tensor.matmul(
                    out=odA[0:64, :bw], lhsT=ones1[:, :], rhs=eb2sb[:, bs:bs + bw],
                    start=True, stop=False, tile_position=(0, 0))
                for pc in range(8):
                    nc.tensor.matmul(
                        out=odA[0:64, :bw], lhsT=zview(pc), rhs=esrc(pc),
                        start=False, stop=(pc == 7), tile_position=(0, 0))
                    nc.tensor.matmul(
                        out=odB[64:128, :bw], lhsT=zview(pc + 8), rhs=esrc(pc + 8),
                        start=(pc == 0), stop=(pc == 7), tile_position=(0, 64))
                osb = osp.tile([64, 512], f32, tag="osb", name=f"osb{nb}")
                nc.scalar.activation(out=osb[:, :bw], in_=odA[0:64, :bw],
                                     func=ACTF.Copy)
                nc.vector.tensor_add(osb[:, :bw], osb[:, :bw], odB[64:128, :bw])
                nc.scalar.dma_start(out=out[:, bs:bs + bw], in_=osb[:, :bw])

    nc.compile()
    return nc


def _host_prep(x0, x1, w1_0, w2_0, gamma0, beta0, w1_1, w2_1, gamma1, beta1,
               shared_w, fc_w, fc_b, embed_w, embed_b, indices):
    import ml_dtypes
    f = np.float32
    bf = ml_dtypes.bfloat16
    x0t = np.ascontiguousarray(x0.transpose(2, 0, 1)).astype(bf)   # [1024, 64, 257]
    x1t = np.ascontiguousarray(x1.transpose(2, 0, 1)).astype(bf)   # [768, 64, 197]
    # w2 transposed, columns grouped in kt-pairs: [KT//2, C, 256]
    w2_0t = w2_0.T.astype(bf)                                      # [C0, H]
    w2_1t = w2_1.T.astype(bf)
    w2_0pp = np.ascontiguousarray(
        w2_0t.reshape(C0, KT // 2, 256).transpose(1, 0, 2))
    w2_1pp = np.ascontiguousarray(
        w2_1t.reshape(C1, KT // 2, 256).transpose(1, 0, 2))
    gbp = np.empty((128, 2, KT), f)
    btp = np.empty((128, 2, KT), f)
    gbp[:, 0, :] = (gamma0 * 0.5).reshape(KT, 128).T
    gbp[:, 1, :] = (gamma1 * 0.5).reshape(KT, 128).T
    btp[:, 0, :] = (beta0 * 0.5).reshape(KT, 128).T
    btp[:, 1, :] = (beta1 * 0.5).reshape(KT, 128).T
    swt = shared_w.T.astype(f)                                    # [1024, 2048]
    fcwt = fc_w.T.astype(f)                                       # [1024, 16384]
    ewt_pad = np.zeros((P, NCORES * W), dtype=bf)
    ewt_pad[:, :KE] = embed_w.T.astype(bf)
    eb2_pad = np.zeros((1, NCORES * W), dtype=bf)
    eb2_pad[0, :KE] = (2.0 * embed_b).astype(bf)

    idx = np.asarray(indices).astype(np.int64)
    in_maps = []
    for i in range(NCORES):
        m = (idx == i).astype(f)
        ew_core = ewt_pad[:, i * W:(i + 1) * W]                   # [2048, W]
        in_maps.append({
            "x0t": np.ascontiguousarray(x0t[:, i * NS:(i + 1) * NS, :]),
            "x1t": np.ascontiguousarray(x1t[:, i * NS:(i + 1) * NS, :]),
            "w2_0p": w2_0pp,
            "w2_1p": w2_1pp,
            "w1_0": np.ascontiguousarray(w1_0, dtype=f),
            "w1_1": np.ascontiguousarray(w1_1, dtype=f),
            "gb": gbp, "bt": btp,
            "cwt": np.ascontiguousarray(swt + fcwt[:, i * P:(i + 1) * P]).astype(bf),
            "fcb": np.ascontiguousarray(fc_b[i * P:(i + 1) * P].reshape(1, P)).astype(bf),
            "maskrow": np.ascontiguousarray(m.reshape(1, N)).astype(bf),
            "mask": np.ascontiguousarray(np.broadcast_to(m, (128, N))),
            "ewt": np.ascontiguousarray(
                ew_core.reshape(PT, 128, W)),
            "eb2": np.ascontiguousarray(eb2_pad[:, i * W:(i + 1) * W]),
        })
    return in_maps


def kernel(**inputs):
    if "/opt/trn_rl_repo" not in sys.path:
        sys.path.insert(0, "/opt/trn_rl_repo")
    from concourse.bass_utils import run_bass_kernel_spmd

    in_maps = _host_prep(**inputs)
    if "nc" not in _CACHE:
        _CACHE["nc"] = _build_nc()
    nc = _CACHE["nc"]
    res = run_bass_kernel_spmd(nc, in_maps, core_ids=list(range(NCORES)))
    outs = [np.asarray(res.results[i]["out"]) for i in range(NCORES)]
    full = np.concatenate(outs, axis=1)[:, :KE]
    return np.ascontiguousarray(full, dtype=np.float32)


if __name__ == "__main__":
    sys.path.insert(0, os.path.dirname(os.path.abspath(__file__)))
    import reference
    inputs = {k: np.asarray(v) for k, v in reference.setup_inputs().items()}
    expected = np.asarray(reference.reference(**inputs))
    actual = kernel(**inputs)
    err = np.abs(actual - expected).max() / (np.abs(expected).max() + 1e-12)
    print("Relative error:", err)


# revision 36
# speedup vs baseline: 1.7811x; 1.0001x over previous
"""Trainium2 Bass kernel for nn_GroupLinearEncoder.

Math (reference):
  h_b = feat_proj(x_b) = BN(einsum over l,c of x_b and w1_b, w2_b)   (N,1024)
  latent = 0.5*(bn(h0)+bn(h1))
  out = (latent @ shared_w.T + subj) @ embed_w.T + 2*embed_b
  where subj = einsum(latent, fc_w[indices]) + b_sel.

Key algebraic folds:
  * group_pred + subj_res share the embed matmul: z = latent@shared_w.T + subj,
    out = z @ embed_w.T + 2*embed_b  -> embed_w is read ONCE.
  * Because every sample belongs to exactly one group, per-core
    cwt_i = shared_w.T + fc_w.T[:, group_i] applied to mask-selected samples
    and AllReduced over cores yields z directly (shared term included).

Sharding over 8 cores:
  * feat_proj: data-parallel over batch (8 samples/core), AllGather h.
  * z: group-parallel (core i handles group i via sample masks), AllReduce (bf16).
  * embed: column-parallel over out_dim (4944 rows/core, padded), concat on host.

Performance structure (vs v1):
  * embed_w (20.3MB/core) is mostly SBUF-resident: 6 p-tiles prefetched during
    stage A, 6 more during the AllGather/BN/C/AllReduce gap (into SBUF freed by
    the x pools), 4 streamed per 512-col block during stage D.
  * stage D matmuls packed 2x via PE column-group tiling (out partitions 0:64
    and 64:128 run concurrently), recovering the M=64 half-array loss.
  * stage A DVE work batched: one mul+reduce per 4-sample PSUM quad tile with
    stride-0 broadcast w1, instead of per-sample ops.
  * BN stats batched per (branch, ktile) via one bn_stats/bn_aggr pair; BN
    affine applied on the Scalar engine (activation scale/bias APs).
  * z AllReduce in bf16 (exact: per column only one core contributes nonzeros).
"""

import os
import sys

import numpy as np

N, H, P, KE = 64, 1024, 2048, 39548
NCORES = 8
NS = N // NCORES            # samples per core
L0, C0 = 257, 1024
L1, C1 = 197, 768
W = 4944                    # embed rows per core (8*4944 = 39552, 4 pad)
PT = P // 128               # 16
KT = H // 128               # 8
NB0 = C0 // 128             # 8
NB1 = C1 // 128             # 6
BN_EPS = 1e-5
EWA = 5                     # ewt p-tiles resident from stage A
EWB = 6                     # ewt p-tiles loaded during the collective gap
NBLK = (W + 511) // 512     # 10 output blocks

_CACHE = {}


def _build_nc():
    if "/opt/trn_rl_repo" not in sys.path:
        sys.path.insert(0, "/opt/trn_rl_repo")
    import concourse.bass as bass
    import concourse.tile as tile
    from concourse import bacc, mybir
    from contextlib import ExitStack

    f32 = mybir.dt.float32
    bf16 = mybir.dt.bfloat16
    ALU = mybir.AluOpType
    ACTF = mybir.ActivationFunctionType
    X = mybir.AxisListType.X

    nc = bacc.Bacc(num_devices=NCORES)

    x0t = nc.declare_dram_parameter("x0t", [C0, NS, L0], bf16, isOutput=False)
    x1t = nc.declare_dram_parameter("x1t", [C1, NS, L1], bf16, isOutput=False)
    # w2 transposed + kt-paired: [KT//2, C, 256] so each DMA line is 512B
    w2_0p = nc.declare_dram_parameter("w2_0p", [KT // 2, C0, 256], bf16, isOutput=False)
    w2_1p = nc.declare_dram_parameter("w2_1p", [KT // 2, C1, 256], bf16, isOutput=False)
    w1_0 = nc.declare_dram_parameter("w1_0", [H, L0], f32, isOutput=False)
    w1_1 = nc.declare_dram_parameter("w1_1", [H, L1], f32, isOutput=False)
    gb = nc.declare_dram_parameter("gb", [128, 2, KT], f32, isOutput=False)
    bt = nc.declare_dram_parameter("bt", [128, 2, KT], f32, isOutput=False)
    cwt = nc.declare_dram_parameter("cwt", [H, P], bf16, isOutput=False)
    fcb = nc.declare_dram_parameter("fcb", [1, P], bf16, isOutput=False)
    maskrow = nc.declare_dram_parameter("maskrow", [1, N], bf16, isOutput=False)
    mask = nc.declare_dram_parameter("mask", [128, N], f32, isOutput=False)
    ewt = nc.declare_dram_parameter("ewt", [PT, 128, W], bf16, isOutput=False)
    eb2 = nc.declare_dram_parameter("eb2", [1, W], bf16, isOutput=False)
    out = nc.declare_dram_parameter("out", [N, W], f32, isOutput=True)

    with tile.TileContext(nc) as tc, ExitStack() as stack:
        singles = stack.enter_context(tc.tile_pool(name="singles", bufs=1))
        dpool = stack.enter_context(tc.tile_pool(name="dram", bufs=1, space="DRAM"))
        tpool = stack.enter_context(tc.tile_pool(name="touchp", bufs=2))
        _tn = [0]

        def touch(ap):
            # absorb a DMA's queue semaphores into DVE's vector clock so
            # downstream DVE ops need only engine-local ordering
            _tn[0] += 1
            tt = tpool.tile([ap.shape[0], 1], ap.dtype, tag="touch",
                            name=f"touch{_tn[0]}")
            nc.vector.tensor_copy(out=tt, in_=ap[:, 0:1])

        # --- persistent small tensors ---
        # hb2 carries h plus per-core partial BN stats through the AllGather:
        # [*, branch, 0:64]=h (col=kt*8+n), [64:72]=per-kt sums, [72:80]=sumsq
        HP = N + 2 * KT                          # 80
        hb2 = singles.tile([128, 2, HP], f32)
        gbsb = singles.tile([128, 2, KT], f32)   # 0.5*gamma, [*, branch, kt]
        btsb = singles.tile([128, 2, KT], f32)   # 0.5*beta
        masksb = singles.tile([128, N], f32)
        epssb = singles.tile([128, 1], f32)
        hall2 = singles.tile([128, NCORES, 2, HP], f32)  # gathered h+stats
        lm_all = singles.tile([128, KT, N], bf16)  # latent*mask per ktile
        zsb_all = singles.tile([128, 2, 8 * N], bf16)    # z after AllReduce
        fcbsb = singles.tile([1, P], bf16)
        maskrsb = singles.tile([1, N], bf16)
        eb2sb = singles.tile([1, W], bf16)
        ones1 = singles.tile([1, N], bf16)

        cwtp = stack.enter_context(tc.tile_pool(name="cwtp", bufs=1))
        ewpA = stack.enter_context(tc.tile_pool(name="ewpA", bufs=1))
        # streamed stage-D chunks live in fresh SBUF (no WAR on the stage-A
        # pools) so their first two blocks load during startup, not at the
        # stage-A -> AllGather boundary where they would delay the h stores
        ewpC = stack.enter_context(tc.tile_pool(name="ewpC", bufs=2))

        # ---- stage-A input pool (freed after stage A; space reused by ewpB)
        ctx_brx = tc.tile_pool(name="brx", bufs=1)
        brx = ctx_brx.__enter__()

        # DMAs on one HWDGE ring execute FIFO (~175GB/s per ring), so the
        # startup-critical loads are split across the sync and gpsimd rings:
        #   sync:   w2_0 pair 0, x0 even ci  (first-matmul dependencies)
        #   gpsimd: x0 odd ci, x1, w1, small singles
        x0sb = [None] * NB0
        x1sb = []
        w2blk = {}

        def _w2_tile(branch, ci, kp):
            return brx.tile([128, 2, 128], bf16, tag=f"w2_{branch}_{ci}", bufs=2,
                            name=f"w2_{branch}_{kp}_{ci}")

        def _load_w2_pair(branch, kp, eng):
            src = w2_0p if branch == 0 else w2_1p
            nb = NB0 if branch == 0 else NB1
            blks = []
            for ci in range(nb):
                t = _w2_tile(branch, ci, kp)
                eng.dma_start(
                    out=t, in_=src[kp, ci * 128:(ci + 1) * 128, :].rearrange(
                        "c (j u) -> c j u", j=2))
                blks.append(t)
            return blks

        w2blk[0] = _load_w2_pair(0, 0, nc.sync)
        for ci in range(NB0):
            t = brx.tile([128, NS, L0], bf16, tag=f"x0_{ci}", name=f"x0_{ci}")
            eng = nc.sync if ci % 2 == 0 else nc.gpsimd
            eng.dma_start(out=t, in_=x0t[ci * 128:(ci + 1) * 128, :, :])
            x0sb[ci] = t
        for ci in range(NB1):
            t = brx.tile([128, NS, L1], bf16, tag=f"x1_{ci}", name=f"x1_{ci}")
            nc.gpsimd.dma_start(out=t, in_=x1t[ci * 128:(ci + 1) * 128, :, :])
            x1sb.append(t)
        w1sb0, w1sb1 = [], []
        for kt in range(KT):
            t = brx.tile([128, L0], f32, tag=f"w10_{kt}", name=f"w10_{kt}")
            nc.gpsimd.dma_start(out=t, in_=w1_0[kt * 128:(kt + 1) * 128, :])
            touch(t)
            w1sb0.append(t)
        for kt in range(KT):
            t = brx.tile([128, L1], f32, tag=f"w11_{kt}", name=f"w11_{kt}")
            nc.gpsimd.dma_start(out=t, in_=w1_1[kt * 128:(kt + 1) * 128, :])
            touch(t)
            w1sb1.append(t)

        nc.gpsimd.dma_start(out=gbsb, in_=gb[:, :, :])
        nc.gpsimd.dma_start(out=btsb, in_=bt[:, :, :])
        nc.gpsimd.dma_start(out=masksb, in_=mask[:, :])
        nc.gpsimd.dma_start(out=fcbsb, in_=fcb[:, :])
        nc.gpsimd.dma_start(out=maskrsb, in_=maskrow[:, :])
        touch(gbsb[:].rearrange("p a b -> p (a b)"))
        touch(btsb[:].rearrange("p a b -> p (a b)"))
        touch(masksb)
        nc.vector.memset(epssb, BN_EPS)
        nc.vector.memset(ones1, 1.0)

        psA_ctx = tc.tile_pool(name="psA", bufs=2, space="PSUM")
        psA = psA_ctx.__enter__()

        # ---------------- stage A : feat_proj ----------------
        cwsb = []
        ewsb = [None] * PT

        # branch 0: 4-sample PSUM quads; one DVE mul+reduce per quad
        for kt in range(KT):
            if kt % 2 == 0 and kt > 0:
                w2blk[0] = _load_w2_pair(0, kt // 2, nc.sync)
            for grp in range(2):
                quad = psA.tile([128, 4, 512], f32, tag="quad",
                                name=f"q0_{kt}_{grp}")
                for ci in range(NB0):
                    lhs = w2blk[0][ci][:, kt % 2, :]
                    for j in range(4):
                        n = grp * 4 + j
                        nc.tensor.matmul(
                            out=quad[:, j, 0:L0],
                            lhsT=lhs,
                            rhs=x0sb[ci][:, n, :],
                            start=(ci == 0),
                            stop=(ci == NB0 - 1),
                        )
                w1b = w1sb0[kt][:].rearrange("p (o l) -> p o l", o=1) \
                    .to_broadcast([128, 4, L0])
                nc.vector.tensor_mul(quad[:, :, 0:L0], quad[:, :, 0:L0], w1b)
                c0 = kt * 8 + grp * 4
                nc.vector.tensor_reduce(
                    out=hb2[:, 0, c0:c0 + 4], in_=quad[:, :, 0:L0],
                    axis=X, op=ALU.add)
            if kt == 2:
                # gate bulk scalar-queue loads behind early stage-A output so
                # x/w2 win HBM bandwidth during startup. Tile schedules by
                # data dependency (not program order), so a dummy write into
                # each destination tile creates the WAW edge that defers the
                # DMA until the gate value exists.
                gsrc = hb2[:, 0, 23:24]
                for kt2 in range(KT):
                    t = cwtp.tile([128, P], bf16, tag=f"cw{kt2}", name=f"cw{kt2}")
                    nc.vector.tensor_copy(out=t[:, 0:1], in_=gsrc)
                    nc.scalar.dma_start(out=t, in_=cwt[kt2 * 128:(kt2 + 1) * 128, :])
                    cwsb.append(t)
                for pc in range(EWA):
                    t = ewpA.tile([128, W], bf16, tag=f"ewA{pc}", name=f"ewA{pc}")
                    nc.vector.tensor_copy(out=t[:, 0:1], in_=gsrc)
                    nc.scalar.dma_start(out=t, in_=ewt[pc, :, :])
                    ewsb[pc] = t
                nc.scalar.dma_start(out=eb2sb, in_=eb2[:, :])

        # branch-0 local BN partial stats (hidden under branch-1 compute)
        w2blk[1] = _load_w2_pair(1, 0, nc.gpsimd)
        sq0 = singles.tile([128, N], f32)

        def _local_stats(b):
            hv = hb2[:, b, 0:N]
            nc.vector.tensor_mul(sq0[:], hv, hv)
            nc.vector.tensor_reduce(
                out=hb2[:, b, N:N + KT],
                in_=hv.rearrange("p (k n) -> p k n", k=KT), axis=X, op=ALU.add)
            nc.vector.tensor_reduce(
                out=hb2[:, b, N + KT:N + 2 * KT],
                in_=sq0[:].rearrange("p (k n) -> p k n", k=KT), axis=X,
                op=ALU.add)

        _local_stats(0)

        # branch 1: two samples per matmul; one DVE mul+reduce per kt
        for kt in range(KT):
            if kt % 2 == 0 and kt > 0:
                w2blk[1] = _load_w2_pair(1, kt // 2, nc.gpsimd)
            quad = psA.tile([128, 4, 512], f32, tag="quad", name=f"q1_{kt}")
            for ci in range(NB1):
                lhs = w2blk[1][ci][:, kt % 2, :]
                for sj in range(4):
                    o = quad[:, sj, :].rearrange("p (s l) -> p s l", s=2)
                    nc.tensor.matmul(
                        out=o[:, :, 0:L1],
                        lhsT=lhs,
                        rhs=x1sb[ci][:, 2 * sj:2 * sj + 2, :],
                        start=(ci == 0),
                        stop=(ci == NB1 - 1),
                    )
            v = quad[:].rearrange("p q (s l) -> p q s l", s=2)[:, :, :, 0:L1]
            w1b = w1sb1[kt][:].rearrange("p (a b l) -> p a b l", a=1, b=1) \
                .to_broadcast([128, 4, 2, L1])
            nc.vector.tensor_mul(v, v, w1b)
            nc.vector.tensor_reduce(
                out=hb2[:, 1, kt * 8:(kt + 1) * 8], in_=v, axis=X, op=ALU.add)

        psA_ctx.__exit__(None, None, None)
        ctx_brx.__exit__(None, None, None)

        # remaining embed weight tiles into the space freed by the x pools;
        # the DMAs wait on the last stage-A reads, then flow during the
        # AllGather / BN / stage-C / AllReduce gap.
        ewpB = stack.enter_context(tc.tile_pool(name="ewpB", bufs=1))
        for pc in range(EWA, EWA + EWB):
            t = ewpB.tile([128, W], bf16, tag=f"ewB{pc}", name=f"ewB{pc}")
            nc.scalar.dma_start(out=t, in_=ewt[pc, :, :])
            ewsb[pc] = t

        # ---------------- stage B : AllGather h+stats + BatchNorm + latent ----------------
        _local_stats(1)
        hbl = dpool.tile([128, 2, HP], f32)
        nc.gpsimd.dma_start(out=hbl[:], in_=hb2[:])
        hgg = dpool.tile([NCORES, 128, 2, HP], f32, addr_space="Shared")
        nc.gpsimd.collective_compute(
            "AllGather",
            ALU.bypass,
            replica_groups=[list(range(NCORES))],
            ins=[hbl[:].opt()],
            outs=[hgg[:].opt()],
        )

        # load gathered h+stats split across both rings (640B segments)
        GH = NCORES // 2
        nc.sync.dma_start(
            out=hall2[:, 0:GH],
            in_=hgg[0:GH].rearrange("g p b c -> p g b c"))
        nc.gpsimd.dma_start(
            out=hall2[:, GH:],
            in_=hgg[GH:].rearrange("g p b c -> p g b c"))
        touch(hall2[:, 0, 0, :])
        touch(hall2[:, GH, 0, :])

        stx = stack.enter_context(tc.tile_pool(name="stats", bufs=2))
        sums = singles.tile([128, 2, KT], f32)     # becomes mean
        sumq = singles.tile([128, 2, KT], f32)     # becomes var
        tmp_ = singles.tile([128, 2, KT], f32)
        # aggregate the per-core partial sums over g
        nc.vector.tensor_reduce(
            out=sums[:],
            in_=hall2[:, :, :, N:N + KT].rearrange("p g b k -> p b k g"),
            axis=X, op=ALU.add)
        nc.vector.tensor_reduce(
            out=sumq[:],
            in_=hall2[:, :, :, N + KT:N + 2 * KT].rearrange(
                "p g b k -> p b k g"),
            axis=X, op=ALU.add)
        # mean/var (biased); rstd = 1/sqrt(var+eps); a = 0.5*gamma*rstd;
        # c = 0.5*beta - mu*a
        nc.vector.tensor_scalar_mul(sums[:], sums[:], 1.0 / N)
        nc.vector.tensor_scalar_mul(sumq[:], sumq[:], 1.0 / N)
        nc.vector.tensor_mul(tmp_[:], sums[:], sums[:])
        nc.vector.tensor_sub(sumq[:], sumq[:], tmp_[:])
        a_ = singles.tile([128, 2, KT], f32)
        c_ = singles.tile([128, 2, KT], f32)
        nc.scalar.activation(out=a_[:], in_=sumq[:], func=ACTF.Sqrt,
                             bias=epssb, scale=1.0)
        nc.vector.reciprocal(out=a_[:].rearrange("p b k -> p (b k)"),
                             in_=a_[:].rearrange("p b k -> p (b k)"))
        nc.vector.tensor_mul(a_[:], a_[:], gbsb[:])
        nc.vector.tensor_mul(c_[:], sums[:], a_[:])
        nc.vector.tensor_sub(c_[:], btsb[:], c_[:])

        # latent = h0*a0 + h1*a1 + (c0+c1), then lm[k, g*8+n] = latent*mask
        lat4 = singles.tile([128, NCORES, KT, NS], f32)
        scr4 = singles.tile([128, NCORES, KT, NS], f32)
        cb = singles.tile([128, KT], f32)

        def _bk4(ap):
            return ap.rearrange("p (a k b) -> p a k b", a=1, b=1) \
                .to_broadcast([128, NCORES, KT, NS])

        nc.vector.tensor_add(cb[:], c_[:, 0, :], c_[:, 1, :])
        KH = KT // 2
        maskb = masksb[:].rearrange("p (o g n) -> p o g n", o=1, g=NCORES) \
            .to_broadcast([128, KH, NCORES, NS])
        for hh in range(2):
            ks = hh * KH

            def _bk4(ap):
                return ap.rearrange("p (a k b) -> p a k b", a=1, b=1) \
                    .to_broadcast([128, NCORES, KH, NS])

            h0v = hall2[:, :, 0, ks * 8:(ks + KH) * 8] \
                .rearrange("p g (k n) -> p g k n", k=KH)
            h1v = hall2[:, :, 1, ks * 8:(ks + KH) * 8] \
                .rearrange("p g (k n) -> p g k n", k=KH)
            l4 = lat4[:, :, ks:ks + KH, :]
            s4 = scr4[:, :, ks:ks + KH, :]
            nc.vector.tensor_mul(l4, h0v, _bk4(a_[:, 0, ks:ks + KH]))
            nc.vector.tensor_mul(s4, h1v, _bk4(a_[:, 1, ks:ks + KH]))
            nc.vector.tensor_add(l4, l4, s4)
            nc.vector.tensor_add(l4, l4, _bk4(cb[:, ks:ks + KH]))
            nc.vector.tensor_mul(
                lm_all[:, ks:ks + KH, :].rearrange("p k (g n) -> p k g n",
                                                   g=NCORES),
                lat4[:, :, ks:ks + KH, :].rearrange("p g k n -> p k g n"),
                maskb)

        # ---------------- stage C : z partial = cwt.T @ lm + fcb x mask ----------------
        zfc_local = dpool.tile([2, 128, 8 * N], bf16)
        with tc.tile_pool(name="zps", bufs=1, space="PSUM") as zps:
            zp = []
            for half in range(2):
                t = zps.tile([128, 8 * N], f32, tag=f"zp{half}", name=f"zp{half}")
                zp.append(t)

            def zpo(pt):
                return zp[pt // 8][:, (pt % 8) * N:(pt % 8 + 1) * N]

            # start=True clears has_written for the WHOLE bank, so only the
            # first matmul touching each bank may set it (kt 0, pt 0/8);
            # later flags=0 matmuls overwrite-where-unset / accumulate-where-set
            for kt in range(KT):
                for pt in range(PT):
                    nc.tensor.matmul(
                        out=zpo(pt),
                        lhsT=cwsb[kt][:, pt * 128:(pt + 1) * 128],
                        rhs=lm_all[:, kt, :],
                        start=(kt == 0 and pt % 8 == 0), stop=False,
                        skip_group_check=True,
                    )
            for pt in range(PT):
                nc.tensor.matmul(
                    out=zpo(pt), lhsT=fcbsb[:, pt * 128:(pt + 1) * 128],
                    rhs=maskrsb[:, :], start=False, stop=True)
            for half in range(2):
                zhb = stx.tile([128, 8 * N], bf16, tag=f"zh{half}", name=f"zh{half}")
                nc.vector.tensor_copy(out=zhb, in_=zp[half][:, :])
                nc.sync.dma_start(out=zfc_local[half], in_=zhb)

        zr = dpool.tile([2, 128, 8 * N], bf16, addr_space="Shared")
        nc.gpsimd.collective_compute(
            "AllReduce",
            ALU.add,
            replica_groups=[list(range(NCORES))],
            ins=[zfc_local[:].opt()],
            outs=[zr[:].opt()],
        )
        nc.sync.dma_start(
            out=zsb_all[:],
            in_=zr[:].rearrange("h p c -> p h c"))
        touch(zsb_all[:, 0, :])

        def zview(pc):
            return zsb_all[:, pc // 8, (pc % 8) * N:(pc % 8 + 1) * N]

        # ---------------- stage D : out = z.T @ ewt + 2*eb ----------------
        with tc.tile_pool(name="odp", bufs=2, space="PSUM") as odp, \
             tc.tile_pool(name="osp", bufs=3) as osp:
            for nb in range(NBLK):
                bs = nb * 512
                bw = min(512, W - bs)
                echunk = {}
                for k, pc in enumerate(range(EWA + EWB, PT)):
                    t = ewpC.tile([128, 512], bf16, tag=f"s{pc}",
                                  name=f"ewC{nb}_{pc}")
                    eng = nc.sync if k % 2 == 0 else nc.gpsimd
                    eng.dma_start(out=t[:, :bw], in_=ewt[pc, :, bs:bs + bw])
                    echunk[pc] = t
                odA = odp.tile([128, 512], f32, tag="odA", name=f"odA{nb}")
                odB = odp.tile([128, 512], f32, tag="odB", name=f"odB{nb}")

                def esrc(pc):
                    if ewsb[pc] is not None:
                        return ewsb[pc][:, bs:bs + bw]
                    return echunk[pc][:, :bw]

                # bias into the lower half's accumulator
                nc.tensor.matmul(
                    out=odA[0:64, :bw], lhsT=ones1[:, :], rhs=eb2sb[:, bs:bs + bw],
                    start=True, stop=False, tile_position=(0, 0))
                for pc in range(8):
                    nc.tensor.matmul(
                        out=odA[0:64, :bw], lhsT=zview(pc), rhs=esrc(pc),
                        start=False, stop=(pc == 7), tile_position=(0, 0))
                    nc.tensor.matmul(
                        out=odB[64:128, :bw], lhsT=zview(pc + 8), rhs=esrc(pc + 8),
                        start=(pc == 0), stop=(pc == 7), tile_position=(0, 64))
                osb = osp.tile([64, 512], f32, tag="osb", name=f"osb{nb}")
                nc.scalar.activation(out=osb[:, :bw], in_=odA[0:64, :bw],
                                     func=ACTF.Copy)
                nc.vector.tensor_add(osb[:, :bw], osb[:, :bw], odB[64:128, :bw])
                nc.scalar.dma_start(out=out[:, bs:bs + bw], in_=osb[:, :bw])

    nc.compile()
    return nc


def _host_prep(x0, x1, w1_0, w2_0, gamma0, beta0, w1_1, w2_1, gamma1, beta1,
               shared_w, fc_w, fc_b, embed_w, embed_b, indices):
    import ml_dtypes
    f = np.float32
    bf = ml_dtypes.bfloat16
    x0t = np.ascontiguousarray(x0.transpose(2, 0, 1)).astype(bf)   # [1024, 64, 257]
    x1t = np.ascontiguousarray(x1.transpose(2, 0, 1)).astype(bf)   # [768, 64, 197]
    # w2 transposed, columns grouped in kt-pairs: [KT//2, C, 256]
    w2_0t = w2_0.T.astype(bf)                                      # [C0, H]
    w2_1t = w2_1.T.astype(bf)
    w2_0pp = np.ascontiguousarray(
        w2_0t.reshape(C0, KT // 2, 256).transpose(1, 0, 2))
    w2_1pp = np.ascontiguousarray(
        w2_1t.reshape(C1, KT // 2, 256).transpose(1, 0, 2))
    gbp = np.empty((128, 2, KT), f)
    btp = np.empty((128, 2, KT), f)
    gbp[:, 0, :] = (gamma0 * 0.5).reshape(KT, 128).T
    gbp[:, 1, :] = (gamma1 * 0.5).reshape(KT, 128).T
    btp[:, 0, :] = (beta0 * 0.5).reshape(KT, 128).T
    btp[:, 1, :] = (beta1 * 0.5).reshape(KT, 128).T
    swt = shared_w.T.astype(f)                                    # [1024, 2048]
    fcwt = fc_w.T.astype(f)                                       # [1024, 16384]
    ewt_pad = np.zeros((P, NCORES * W), dtype=bf)
    ewt_pad[:, :KE] = embed_w.T.astype(bf)
    eb2_pad = np.zeros((1, NCORES * W), dtype=bf)
    eb2_pad[0, :KE] = (2.0 * embed_b).astype(bf)

    idx = np.asarray(indices).astype(np.int64)
    in_maps = []
    for i in range(NCORES):
        m = (idx == i).astype(f)
        ew_core = ewt_pad[:, i * W:(i + 1) * W]                   # [2048, W]
        in_maps.append({
            "x0t": np.ascontiguousarray(x0t[:, i * NS:(i + 1) * NS, :]),
            "x1t": np.ascontiguousarray(x1t[:, i * NS:(i + 1) * NS, :]),
            "w2_0p": w2_0pp,
            "w2_1p": w2_1pp,
            "w1_0": np.ascontiguousarray(w1_0, dtype=f),
            "w1_1": np.ascontiguousarray(w1_1, dtype=f),
            "gb": gbp, "bt": btp,
            "cwt": np.ascontiguousarray(swt + fcwt[:, i * P:(i + 1) * P]).astype(bf),
            "fcb": np.ascontiguousarray(fc_b[i * P:(i + 1) * P].reshape(1, P)).astype(bf),
            "maskrow": np.ascontiguousarray(m.reshape(1, N)).astype(bf),
            "mask": np.ascontiguousarray(np.broadcast_to(m, (128, N))),
            "ewt": np.ascontiguousarray(
                ew_core.reshape(PT, 128, W)),
            "eb2": np.ascontiguousarray(eb2_pad[:, i * W:(i + 1) * W]),
        })
    return in_maps


def kernel(**inputs):
    if "/opt/trn_rl_repo" not in sys.path:
        sys.path.insert(0, "/opt/trn_rl_repo")
    from concourse.bass_utils import run_bass_kernel_spmd

    in_maps = _host_prep(**inputs)
    if "nc" not in _CACHE:
        _CACHE["nc"] = _build_nc()
    nc = _CACHE["nc"]
    res = run_bass_kernel_spmd(nc, in_maps, core_ids=list(range(NCORES)))
    outs = [np.asarray(res.results[i]["out"]) for i in range(NCORES)]
    full = np.concatenate(outs, axis=1)[:, :KE]
    return np.ascontiguousarray(full, dtype=np.float32)


if __name__ == "__main__":
    sys.path.insert(0, os.path.dirname(os.path.abspath(__file__)))
    import reference
    inputs = {k: np.asarray(v) for k, v in reference.setup_inputs().items()}
    expected = np.asarray(reference.reference(**inputs))
    actual = kernel(**inputs)
    err = np.abs(actual - expected).max() / (np.abs(expected).max() + 1e-12)
    print("Relative error:", err)


# revision 50
# speedup vs baseline: 1.8250x; 1.0246x over previous
"""Trainium2 Bass kernel for nn_GroupLinearEncoder.

Math (reference):
  h_b = feat_proj(x_b) = BN(einsum over l,c of x_b and w1_b, w2_b)   (N,1024)
  latent = 0.5*(bn(h0)+bn(h1))
  out = (latent @ shared_w.T + subj) @ embed_w.T + 2*embed_b
  where subj = einsum(latent, fc_w[indices]) + b_sel.

Key algebraic folds:
  * group_pred + subj_res share the embed matmul: z = latent@shared_w.T + subj,
    out = z @ embed_w.T + 2*embed_b  -> embed_w is read ONCE.
  * Because every sample belongs to exactly one group, per-core
    cwt_i = shared_w.T + fc_w.T[:, group_i] applied to mask-selected samples
    and AllReduced over cores yields z directly (shared term included).

Sharding over 8 cores:
  * feat_proj: data-parallel over batch (8 samples/core), AllGather h.
  * z: group-parallel (core i handles group i via sample masks), AllReduce (bf16).
  * embed: column-parallel over out_dim (4944 rows/core, padded), concat on host.

Performance structure (vs v1):
  * embed_w (20.3MB/core) is mostly SBUF-resident: 6 p-tiles prefetched during
    stage A, 6 more during the AllGather/BN/C/AllReduce gap (into SBUF freed by
    the x pools), 4 streamed per 512-col block during stage D.
  * stage D matmuls packed 2x via PE column-group tiling (out partitions 0:64
    and 64:128 run concurrently), recovering the M=64 half-array loss.
  * stage A DVE work batched: one mul+reduce per 4-sample PSUM quad tile with
    stride-0 broadcast w1, instead of per-sample ops.
  * BN stats batched per (branch, ktile) via one bn_stats/bn_aggr pair; BN
    affine applied on the Scalar engine (activation scale/bias APs).
  * z AllReduce in bf16 (exact: per column only one core contributes nonzeros).
"""

import os
import sys

import numpy as np

N, H, P, KE = 64, 1024, 2048, 39548
NCORES = 8
NS = N // NCORES            # samples per core
L0, C0 = 257, 1024
L1, C1 = 197, 768
W = 4944                    # embed rows per core (8*4944 = 39552, 4 pad)
PT = P // 128               # 16
KT = H // 128               # 8
NB0 = C0 // 128             # 8
NB1 = C1 // 128             # 6
BN_EPS = 1e-5
EWA = 5                     # ewt p-tiles resident from stage A
EWB = 6                     # ewt p-tiles loaded during the collective gap
NBLK = (W + 511) // 512     # 10 output blocks

_CACHE = {}


def _build_nc():
    if "/opt/trn_rl_repo" not in sys.path:
        sys.path.insert(0, "/opt/trn_rl_repo")
    import concourse.bass as bass
    import concourse.tile as tile
    from concourse import bacc, mybir
    from contextlib import ExitStack

    f32 = mybir.dt.float32
    bf16 = mybir.dt.bfloat16
    ALU = mybir.AluOpType
    ACTF = mybir.ActivationFunctionType
    X = mybir.AxisListType.X

    nc = bacc.Bacc(num_devices=NCORES)

    x0t = nc.declare_dram_parameter("x0t", [C0, NS, L0], bf16, isOutput=False)
    x1t = nc.declare_dram_parameter("x1t", [C1, NS, L1], bf16, isOutput=False)
    # w2 transposed + kt-paired: [KT//2, C, 256] so each DMA line is 512B
    w2_0p = nc.declare_dram_parameter("w2_0p", [KT // 2, C0, 256], bf16, isOutput=False)
    w2_1p = nc.declare_dram_parameter("w2_1p", [KT // 2, C1, 256], bf16, isOutput=False)
    w1_0 = nc.declare_dram_parameter("w1_0", [H, L0], f32, isOutput=False)
    w1_1 = nc.declare_dram_parameter("w1_1", [H, L1], f32, isOutput=False)
    gb = nc.declare_dram_parameter("gb", [128, 2, KT], f32, isOutput=False)
    bt = nc.declare_dram_parameter("bt", [128, 2, KT], f32, isOutput=False)
    cwt = nc.declare_dram_parameter("cwt", [H, P], bf16, isOutput=False)
    fcb = nc.declare_dram_parameter("fcb", [1, P], bf16, isOutput=False)
    maskrow = nc.declare_dram_parameter("maskrow", [1, N], bf16, isOutput=False)
    mask = nc.declare_dram_parameter("mask", [128, N], f32, isOutput=False)
    ewt = nc.declare_dram_parameter("ewt", [PT, 128, W], bf16, isOutput=False)
    eb2 = nc.declare_dram_parameter("eb2", [1, W], bf16, isOutput=False)
    out = nc.declare_dram_parameter("out", [N, W], f32, isOutput=True)

    with tile.TileContext(nc) as tc, ExitStack() as stack:
        singles = stack.enter_context(tc.tile_pool(name="singles", bufs=1))
        dpool = stack.enter_context(tc.tile_pool(name="dram", bufs=1, space="DRAM"))
        tpool = stack.enter_context(tc.tile_pool(name="touchp", bufs=2))
        _tn = [0]

        def touch(ap):
            # absorb a DMA's queue semaphores into DVE's vector clock so
            # downstream DVE ops need only engine-local ordering
            _tn[0] += 1
            tt = tpool.tile([ap.shape[0], 1], ap.dtype, tag="touch",
                            name=f"touch{_tn[0]}")
            nc.vector.tensor_copy(out=tt, in_=ap[:, 0:1])

        # --- persistent small tensors ---
        # hb2 carries h plus per-core partial BN stats through the AllGather:
        # [*, branch, 0:64]=h (col=kt*8+n), [64:72]=per-kt sums, [72:80]=sumsq
        HP = N + 2 * KT                          # 80
        hb2 = singles.tile([128, 2, HP], f32)
        gbsb = singles.tile([128, 2, KT], f32)   # 0.5*gamma, [*, branch, kt]
        btsb = singles.tile([128, 2, KT], f32)   # 0.5*beta
        masksb = singles.tile([128, N], f32)
        epssb = singles.tile([128, 1], f32)
        hstat = singles.tile([128, NCORES, 2, 2 * KT], f32)  # gathered stats
        hallh = singles.tile([128, NCORES, 2, N], f32)       # gathered h
        # latent*mask per ktile, split in two tiles so stage C's first
        # matmuls depend only on the first half (Tile tracks deps per tile)
        lm_half = [singles.tile([128, KT // 2, N], bf16, tag=f"lmh{h}",
                                name=f"lmh{h}") for h in range(2)]

        def lm_kt(kt):
            return lm_half[kt // (KT // 2)][:, kt % (KT // 2), :]
        zsb_all = singles.tile([128, 2, 8 * N], bf16)    # z after AllReduce
        fcbsb = singles.tile([1, P], bf16)
        maskrsb = singles.tile([1, N], bf16)
        eb2sb = singles.tile([1, W], bf16)
        ones1 = singles.tile([1, N], bf16)

        cwtp = stack.enter_context(tc.tile_pool(name="cwtp", bufs=1))
        ewpA = stack.enter_context(tc.tile_pool(name="ewpA", bufs=1))
        # streamed stage-D chunks live in fresh SBUF (no WAR on the stage-A
        # pools) so their first two blocks load during startup, not at the
        # stage-A -> AllGather boundary where they would delay the h stores
        ewpC = stack.enter_context(tc.tile_pool(name="ewpC", bufs=2))

        # ---- stage-A input pool (freed after stage A; space reused by ewpB)
        ctx_brx = tc.tile_pool(name="brx", bufs=1)
        brx = ctx_brx.__enter__()

        # DMAs on one HWDGE ring execute FIFO (~175GB/s per ring), so the
        # startup-critical loads are split across the sync and gpsimd rings:
        #   sync:   w2_0 pair 0, x0 even ci  (first-matmul dependencies)
        #   gpsimd: x0 odd ci, x1, w1, small singles
        x0sb = [None] * NB0
        x1sb = []
        w2blk = {}

        def _w2_tile(branch, ci, kp):
            return brx.tile([128, 2, 128], bf16, tag=f"w2_{branch}_{ci}", bufs=2,
                            name=f"w2_{branch}_{kp}_{ci}")

        def _load_w2_pair(branch, kp, eng):
            src = w2_0p if branch == 0 else w2_1p
            nb = NB0 if branch == 0 else NB1
            blks = []
            for ci in range(nb):
                t = _w2_tile(branch, ci, kp)
                eng.dma_start(
                    out=t, in_=src[kp, ci * 128:(ci + 1) * 128, :].rearrange(
                        "c (j u) -> c j u", j=2))
                blks.append(t)
            return blks

        w2blk[0] = _load_w2_pair(0, 0, nc.sync)
        w2blk0_p1 = _load_w2_pair(0, 1, nc.sync)
        for ci in range(NB0):
            t = brx.tile([128, NS, L0], bf16, tag=f"x0_{ci}", name=f"x0_{ci}")
            eng = nc.sync if ci % 2 == 0 else nc.gpsimd
            eng.dma_start(out=t, in_=x0t[ci * 128:(ci + 1) * 128, :, :])
            x0sb[ci] = t
        for ci in range(NB1):
            t = brx.tile([128, NS, L1], bf16, tag=f"x1_{ci}", name=f"x1_{ci}")
            nc.gpsimd.dma_start(out=t, in_=x1t[ci * 128:(ci + 1) * 128, :, :])
            x1sb.append(t)
        w1sb0, w1sb1 = [], []
        for kt in range(KT):
            t = brx.tile([128, L0], f32, tag=f"w10_{kt}", name=f"w10_{kt}")
            nc.gpsimd.dma_start(out=t, in_=w1_0[kt * 128:(kt + 1) * 128, :])
            touch(t)
            w1sb0.append(t)
        for kt in range(KT):
            t = brx.tile([128, L1], f32, tag=f"w11_{kt}", name=f"w11_{kt}")
            nc.gpsimd.dma_start(out=t, in_=w1_1[kt * 128:(kt + 1) * 128, :])
            touch(t)
            w1sb1.append(t)

        nc.gpsimd.dma_start(out=gbsb, in_=gb[:, :, :])
        nc.gpsimd.dma_start(out=btsb, in_=bt[:, :, :])
        nc.gpsimd.dma_start(out=masksb, in_=mask[:, :])
        nc.gpsimd.dma_start(out=fcbsb, in_=fcb[:, :])
        nc.gpsimd.dma_start(out=maskrsb, in_=maskrow[:, :])
        touch(gbsb[:].rearrange("p a b -> p (a b)"))
        touch(btsb[:].rearrange("p a b -> p (a b)"))
        touch(masksb)
        nc.vector.memset(epssb, BN_EPS)
        nc.vector.memset(ones1, 1.0)

        psA_ctx = tc.tile_pool(name="psA", bufs=2, space="PSUM")
        psA = psA_ctx.__enter__()

        # ---------------- stage A : feat_proj ----------------
        cwsb = []
        ewsb = [None] * PT

        # branch 0: 4-sample PSUM quads; one DVE mul+reduce per quad
        for kt in range(KT):
            if kt % 2 == 0 and kt > 0:
                w2blk[0] = w2blk0_p1 if kt == 2 else \
                    _load_w2_pair(0, kt // 2, nc.sync)
            for grp in range(2):
                quad = psA.tile([128, 4, 512], f32, tag="quad",
                                name=f"q0_{kt}_{grp}")
                for ci in range(NB0):
                    lhs = w2blk[0][ci][:, kt % 2, :]
                    for j in range(4):
                        n = grp * 4 + j
                        nc.tensor.matmul(
                            out=quad[:, j, 0:L0],
                            lhsT=lhs,
                            rhs=x0sb[ci][:, n, :],
                            start=(ci == 0),
                            stop=(ci == NB0 - 1),
                        )
                w1b = w1sb0[kt][:].rearrange("p (o l) -> p o l", o=1) \
                    .to_broadcast([128, 4, L0])
                nc.vector.tensor_mul(quad[:, :, 0:L0], quad[:, :, 0:L0], w1b)
                c0 = kt * 8 + grp * 4
                nc.vector.tensor_reduce(
                    out=hb2[:, 0, c0:c0 + 4], in_=quad[:, :, 0:L0],
                    axis=X, op=ALU.add)
            if kt == 2:
                # gate bulk scalar-queue loads behind early stage-A output so
                # x/w2 win HBM bandwidth during startup. Tile schedules by
                # data dependency (not program order), so a dummy write into
                # each destination tile creates the WAW edge that defers the
                # DMA until the gate value exists.
                gsrc = hb2[:, 0, 23:24]
                for kt2 in range(KT):
                    t = cwtp.tile([128, P], bf16, tag=f"cw{kt2}", name=f"cw{kt2}")
                    nc.vector.tensor_copy(out=t[:, 0:1], in_=gsrc)
                    nc.scalar.dma_start(out=t, in_=cwt[kt2 * 128:(kt2 + 1) * 128, :])
                    cwsb.append(t)
                for pc in range(EWA):
                    t = ewpA.tile([128, W], bf16, tag=f"ewA{pc}", name=f"ewA{pc}")
                    nc.vector.tensor_copy(out=t[:, 0:1], in_=gsrc)
                    nc.scalar.dma_start(out=t, in_=ewt[pc, :, :])
                    ewsb[pc] = t
                nc.scalar.dma_start(out=eb2sb, in_=eb2[:, :])

        # branch-0 local BN partial stats (hidden under branch-1 compute)
        w2blk[1] = _load_w2_pair(1, 0, nc.sync)
        sq0 = singles.tile([128, N], f32)

        def _local_stats(b):
            hv = hb2[:, b, 0:N]
            nc.vector.tensor_mul(sq0[:], hv, hv)
            nc.vector.tensor_reduce(
                out=hb2[:, b, N:N + KT],
                in_=hv.rearrange("p (k n) -> p k n", k=KT), axis=X, op=ALU.add)
            nc.vector.tensor_reduce(
                out=hb2[:, b, N + KT:N + 2 * KT],
                in_=sq0[:].rearrange("p (k n) -> p k n", k=KT), axis=X,
                op=ALU.add)

        _local_stats(0)
        # store the branch-0 half of the AllGather payload early
        hbl = dpool.tile([128, 2, HP], f32)
        nc.gpsimd.dma_start(out=hbl[:, 0], in_=hb2[:, 0])

        # warmup collective: paced to finish shortly before the real
        # AllGather so the collectives firmware is awake when it triggers
        # (a cold trigger costs ~11us). Gated on branch-1 kt=2 output.
        wdin = dpool.tile([1, 8], f32)
        wdout = dpool.tile([NCORES, 1, 8], f32, addr_space="Shared")

        # branch 1: two samples per matmul; one DVE mul+reduce per kt
        for kt in range(KT):
            if kt % 2 == 0 and kt > 0:
                w2blk[1] = _load_w2_pair(1, kt // 2, nc.sync)
            quad = psA.tile([128, 4, 512], f32, tag="quad", name=f"q1_{kt}")
            for ci in range(NB1):
                lhs = w2blk[1][ci][:, kt % 2, :]
                for sj in range(4):
                    o = quad[:, sj, :].rearrange("p (s l) -> p s l", s=2)
                    nc.tensor.matmul(
                        out=o[:, :, 0:L1],
                        lhsT=lhs,
                        rhs=x1sb[ci][:, 2 * sj:2 * sj + 2, :],
                        start=(ci == 0),
                        stop=(ci == NB1 - 1),
                    )
            v = quad[:].rearrange("p q (s l) -> p q s l", s=2)[:, :, :, 0:L1]
            w1b4 = w1sb1[kt][:].rearrange("p (a b l) -> p a b l", a=1, b=1) \
                .to_broadcast([128, 4, 2, L1])
            if kt < KT - 1:
                nc.vector.tensor_mul(v, v, w1b4)
                nc.vector.tensor_reduce(
                    out=hb2[:, 1, kt * 8:(kt + 1) * 8], in_=v, axis=X,
                    op=ALU.add)
            else:
                # split the tail kt's DVE so the last piece is small and the
                # h store can fire right after the last matmul drains
                w1b1 = w1sb1[kt][:].rearrange("p (a b l) -> p a b l",
                                              a=1, b=1) \
                    .to_broadcast([128, 1, 2, L1])
                for sj in range(4):
                    vs = v[:, sj:sj + 1]
                    nc.vector.tensor_mul(vs, vs, w1b1)
                    nc.vector.tensor_reduce(
                        out=hb2[:, 1, kt * 8 + 2 * sj:kt * 8 + 2 * sj + 2],
                        in_=vs, axis=X, op=ALU.add)
            if kt == 2:
                nc.gpsimd.dma_start(out=wdin, in_=hb2[0:1, 1, 16:24])
                nc.gpsimd.collective_compute(
                    "AllGather",
                    ALU.bypass,
                    replica_groups=[list(range(NCORES))],
                    ins=[wdin[:].opt()],
                    outs=[wdout[:].opt()],
                )

        psA_ctx.__exit__(None, None, None)
        ctx_brx.__exit__(None, None, None)

        # remaining embed weight tiles into the space freed by the x pools;
        # the DMAs wait on the last stage-A reads, then flow during the
        # AllGather / BN / stage-C / AllReduce gap.
        ewpB = stack.enter_context(tc.tile_pool(name="ewpB", bufs=1))
        for pc in range(EWA, EWA + EWB):
            t = ewpB.tile([128, W], bf16, tag=f"ewB{pc}", name=f"ewB{pc}")
            nc.scalar.dma_start(out=t, in_=ewt[pc, :, :])
            ewsb[pc] = t

        # ---------------- stage B : AllGather h+stats + BatchNorm + latent ----------------
        _local_stats(1)
        nc.gpsimd.dma_start(out=hbl[:, 1], in_=hb2[:, 1])
        hgg = dpool.tile([NCORES, 128, 2, HP], f32, addr_space="Shared")
        nc.gpsimd.collective_compute(
            "AllGather",
            ALU.bypass,
            replica_groups=[list(range(NCORES))],
            ins=[hbl[:].opt()],
            outs=[hgg[:].opt()],
        )

        # stats first (small, gates the BN arithmetic), then the h columns
        GH = NCORES // 2
        for b in range(2):
            nc.sync.dma_start(
                out=hstat[:, :, b, :],
                in_=hgg[:, :, b, N:].rearrange("g p c -> p g c"))
        for b in range(2):
            nc.sync.dma_start(
                out=hallh[:, 0:GH, b, :],
                in_=hgg[0:GH, :, b, 0:N].rearrange("g p c -> p g c"))
            nc.gpsimd.dma_start(
                out=hallh[:, GH:, b, :],
                in_=hgg[GH:, :, b, 0:N].rearrange("g p c -> p g c"))
        touch(hstat[:, 0, 0, :])
        touch(hallh[:, 0, 0, :])
        touch(hallh[:, GH, 0, :])

        stx = stack.enter_context(tc.tile_pool(name="stats", bufs=2))
        sums = singles.tile([128, 2, KT], f32)     # becomes mean
        sumq = singles.tile([128, 2, KT], f32)     # becomes var
        tmp_ = singles.tile([128, 2, KT], f32)
        # aggregate the per-core partial sums over g
        nc.vector.tensor_reduce(
            out=sums[:],
            in_=hstat[:, :, :, 0:KT].rearrange("p g b k -> p b k g"),
            axis=X, op=ALU.add)
        nc.vector.tensor_reduce(
            out=sumq[:],
            in_=hstat[:, :, :, KT:2 * KT].rearrange("p g b k -> p b k g"),
            axis=X, op=ALU.add)
        # mean/var (biased); rstd = 1/sqrt(var+eps); a = 0.5*gamma*rstd;
        # c = 0.5*beta - mu*a
        nc.vector.tensor_scalar_mul(sums[:], sums[:], 1.0 / N)
        nc.vector.tensor_scalar_mul(sumq[:], sumq[:], 1.0 / N)
        nc.vector.tensor_mul(tmp_[:], sums[:], sums[:])
        nc.vector.tensor_sub(sumq[:], sumq[:], tmp_[:])
        a_ = singles.tile([128, 2, KT], f32)
        c_ = singles.tile([128, 2, KT], f32)
        nc.scalar.activation(out=a_[:], in_=sumq[:], func=ACTF.Sqrt,
                             bias=epssb, scale=1.0)
        nc.vector.reciprocal(out=a_[:].rearrange("p b k -> p (b k)"),
                             in_=a_[:].rearrange("p b k -> p (b k)"))
        nc.vector.tensor_mul(a_[:], a_[:], gbsb[:])
        nc.vector.tensor_mul(c_[:], sums[:], a_[:])
        nc.vector.tensor_sub(c_[:], btsb[:], c_[:])

        # latent = h0*a0 + h1*a1 + (c0+c1), then lm[k, g*8+n] = latent*mask
        lat4 = singles.tile([128, NCORES, KT, NS], f32)
        scr4 = singles.tile([128, NCORES, KT, NS], f32)
        cb = singles.tile([128, KT], f32)

        def _bk4(ap):
            return ap.rearrange("p (a k b) -> p a k b", a=1, b=1) \
                .to_broadcast([128, NCORES, KT, NS])

        nc.vector.tensor_add(cb[:], c_[:, 0, :], c_[:, 1, :])
        KH = KT // 2
        maskb = masksb[:].rearrange("p (o g n) -> p o g n", o=1, g=NCORES) \
            .to_broadcast([128, KH, NCORES, NS])
        for hh in range(2):
            ks = hh * KH

            def _bk4(ap):
                return ap.rearrange("p (a k b) -> p a k b", a=1, b=1) \
                    .to_broadcast([128, NCORES, KH, NS])

            h0v = hallh[:, :, 0, ks * 8:(ks + KH) * 8] \
                .rearrange("p g (k n) -> p g k n", k=KH)
            h1v = hallh[:, :, 1, ks * 8:(ks + KH) * 8] \
                .rearrange("p g (k n) -> p g k n", k=KH)
            l4 = lat4[:, :, ks:ks + KH, :]
            s4 = scr4[:, :, ks:ks + KH, :]
            nc.vector.tensor_mul(l4, h0v, _bk4(a_[:, 0, ks:ks + KH]))
            nc.vector.tensor_mul(s4, h1v, _bk4(a_[:, 1, ks:ks + KH]))
            nc.vector.tensor_add(l4, l4, s4)
            nc.vector.tensor_add(l4, l4, _bk4(cb[:, ks:ks + KH]))
            nc.vector.tensor_mul(
                lm_half[hh][:].rearrange("p k (g n) -> p k g n", g=NCORES),
                lat4[:, :, ks:ks + KH, :].rearrange("p g k n -> p k g n"),
                maskb)

        # ---------------- stage C : z partial = cwt.T @ lm + fcb x mask ----------------
        zfc_local = dpool.tile([2, 128, 8 * N], bf16)
        with tc.tile_pool(name="zps", bufs=1, space="PSUM") as zps:
            zp = []
            for half in range(2):
                t = zps.tile([128, 8 * N], f32, tag=f"zp{half}", name=f"zp{half}")
                zp.append(t)

            def zpo(pt):
                return zp[pt // 8][:, (pt % 8) * N:(pt % 8 + 1) * N]

            # start=True clears has_written for the WHOLE bank, so only the
            # first matmul touching each bank may set it (kt 0, pt 0/8);
            # later flags=0 matmuls overwrite-where-unset / accumulate-where-set
            for kt in range(KT):
                for pt in range(PT):
                    nc.tensor.matmul(
                        out=zpo(pt),
                        lhsT=cwsb[kt][:, pt * 128:(pt + 1) * 128],
                        rhs=lm_kt(kt),
                        start=(kt == 0 and pt % 8 == 0), stop=False,
                        skip_group_check=True,
                    )
            for pt in range(PT):
                nc.tensor.matmul(
                    out=zpo(pt), lhsT=fcbsb[:, pt * 128:(pt + 1) * 128],
                    rhs=maskrsb[:, :], start=False, stop=True)
            for half in range(2):
                zhb = stx.tile([128, 8 * N], bf16, tag=f"zh{half}", name=f"zh{half}")
                nc.vector.tensor_copy(out=zhb, in_=zp[half][:, :])
                nc.sync.dma_start(out=zfc_local[half], in_=zhb)

        zr = dpool.tile([2, 128, 8 * N], bf16, addr_space="Shared")
        nc.gpsimd.collective_compute(
            "AllReduce",
            ALU.add,
            replica_groups=[list(range(NCORES))],
            ins=[zfc_local[:].opt()],
            outs=[zr[:].opt()],
        )
        nc.sync.dma_start(out=zsb_all[:, 0], in_=zr[0])
        nc.gpsimd.dma_start(out=zsb_all[:, 1], in_=zr[1])
        touch(zsb_all[:, 0, :])
        touch(zsb_all[:, 1, :])

        def zview(pc):
            return zsb_all[:, pc // 8, (pc % 8) * N:(pc % 8 + 1) * N]

        # ---------------- stage D : out = z.T @ ewt + 2*eb ----------------
        with tc.tile_pool(name="odp", bufs=2, space="PSUM") as odp, \
             tc.tile_pool(name="osp", bufs=3) as osp:
            for nb in range(NBLK):
                bs = nb * 512
                bw = min(512, W - bs)
                echunk = {}
                for k, pc in enumerate(range(EWA + EWB, PT)):
                    t = ewpC.tile([128, 512], bf16, tag=f"s{pc}",
                                  name=f"ewC{nb}_{pc}")
                    eng = nc.sync if k % 2 == 0 else nc.gpsimd
                    eng.dma_start(out=t[:, :bw], in_=ewt[pc, :, bs:bs + bw])
                    echunk[pc] = t
                odA = odp.tile([128, 512], f32, tag="odA", name=f"odA{nb}")
                odB = odp.tile([128, 512], f32, tag="odB", name=f"odB{nb}")

                def esrc(pc):
                    if ewsb[pc] is not None:
                        return ewsb[pc][:, bs:bs + bw]
                    return echunk[pc][:, :bw]

                # bias into the lower half's accumulator
                nc.tensor.matmul(
                    out=odA[0:64, :bw], lhsT=ones1[:, :], rhs=eb2sb[:, bs:bs + bw],
                    start=True, stop=False, tile_position=(0, 0))
                for pc in range(8):
                    nc.tensor.matmul(
                        out=odA[0:64, :bw], lhsT=zview(pc), rhs=esrc(pc),
                        start=False, stop=(pc == 7), tile_position=(0, 0))
                    nc.tensor.matmul(
                        out=odB[64:128, :bw], lhsT=zview(pc + 8), rhs=esrc(pc + 8),
                        start=(pc == 0), stop=(pc == 7), tile_position=(0, 64))
                osb = osp.tile([64, 512], f32, tag="osb", name=f"osb{nb}")
                nc.scalar.activation(out=osb[:, :bw], in_=odA[0:64, :bw],
                                     func=ACTF.Copy)
                nc.vector.tensor_add(osb[:, :bw], osb[:, :bw], odB[64:128, :bw])
                nc.scalar.dma_start(out=out[:, bs:bs + bw], in_=osb[:, :bw])

    nc.compile()
    return nc


def _host_prep(x0, x1, w1_0, w2_0, gamma0, beta0, w1_1, w2_1, gamma1, beta1,
               shared_w, fc_w, fc_b, embed_w, embed_b, indices):
    import ml_dtypes
    f = np.float32
    bf = ml_dtypes.bfloat16
    x0t = np.ascontiguousarray(x0.transpose(2, 0, 1)).astype(bf)   # [1024, 64, 257]
    x1t = np.ascontiguousarray(x1.transpose(2, 0, 1)).astype(bf)   # [768, 64, 197]
    # w2 transposed, columns grouped in kt-pairs: [KT//2, C, 256]
    w2_0t = w2_0.T.astype(bf)                                      # [C0, H]
    w2_1t = w2_1.T.astype(bf)
    w2_0pp = np.ascontiguousarray(
        w2_0t.reshape(C0, KT // 2, 256).transpose(1, 0, 2))
    w2_1pp = np.ascontiguousarray(
        w2_1t.reshape(C1, KT // 2, 256).transpose(1, 0, 2))
    gbp = np.empty((128, 2, KT), f)
    btp = np.empty((128, 2, KT), f)
    gbp[:, 0, :] = (gamma0 * 0.5).reshape(KT, 128).T
    gbp[:, 1, :] = (gamma1 * 0.5).reshape(KT, 128).T
    btp[:, 0, :] = (beta0 * 0.5).reshape(KT, 128).T
    btp[:, 1, :] = (beta1 * 0.5).reshape(KT, 128).T
    swt = shared_w.T.astype(f)                                    # [1024, 2048]
    fcwt = fc_w.T.astype(f)                                       # [1024, 16384]
    ewt_pad = np.zeros((P, NCORES * W), dtype=bf)
    ewt_pad[:, :KE] = embed_w.T.astype(bf)
    eb2_pad = np.zeros((1, NCORES * W), dtype=bf)
    eb2_pad[0, :KE] = (2.0 * embed_b).astype(bf)

    idx = np.asarray(indices).astype(np.int64)
    in_maps = []
    for i in range(NCORES):
        m = (idx == i).astype(f)
        ew_core = ewt_pad[:, i * W:(i + 1) * W]                   # [2048, W]
        in_maps.append({
            "x0t": np.ascontiguousarray(x0t[:, i * NS:(i + 1) * NS, :]),
            "x1t": np.ascontiguousarray(x1t[:, i * NS:(i + 1) * NS, :]),
            "w2_0p": w2_0pp,
            "w2_1p": w2_1pp,
            "w1_0": np.ascontiguousarray(w1_0, dtype=f),
            "w1_1": np.ascontiguousarray(w1_1, dtype=f),
            "gb": gbp, "bt": btp,
            "cwt": np.ascontiguousarray(swt + fcwt[:, i * P:(i + 1) * P]).astype(bf),
            "fcb": np.ascontiguousarray(fc_b[i * P:(i + 1) * P].reshape(1, P)).astype(bf),
            "maskrow": np.ascontiguousarray(m.reshape(1, N)).astype(bf),
            "mask": np.ascontiguousarray(np.broadcast_to(m, (128, N))),
            "ewt": np.ascontiguousarray(
                ew_core.reshape(PT, 128, W)),
            "eb2": np.ascontiguousarray(eb2_pad[:, i * W:(i + 1) * W]),
        })
    return in_maps


def kernel(**inputs):
    if "/opt/trn_rl_repo" not in sys.path:
        sys.path.insert(0, "/opt/trn_rl_repo")
    from concourse.bass_utils import run_bass_kernel_spmd

    in_maps = _host_prep(**inputs)
    if "nc" not in _CACHE:
        _CACHE["nc"] = _build_nc()
    nc = _CACHE["nc"]
    res = run_bass_kernel_spmd(nc, in_maps, core_ids=list(range(NCORES)))
    outs = [np.asarray(res.results[i]["out"]) for i in range(NCORES)]
    full = np.concatenate(outs, axis=1)[:, :KE]
    return np.ascontiguousarray(full, dtype=np.float32)


if __name__ == "__main__":
    sys.path.insert(0, os.path.dirname(os.path.abspath(__file__)))
    import reference
    inputs = {k: np.asarray(v) for k, v in reference.setup_inputs().items()}
    expected = np.asarray(reference.reference(**inputs))
    actual = kernel(**inputs)
    err = np.abs(actual - expected).max() / (np.abs(expected).max() + 1e-12)
    print("Relative error:", err)
